# revision 19
# baseline (speedup 1.0000x reference)
"""Trainium2 Bass kernel for nn_LocalCrossModalAttention (sparse k-NN cross attention).

Sharding: rows (queries) L=3072 split across 8 cores (384 each). K/V projections
(with Wo folded into V) are computed row-parallel, AllGathered as a bf16
[3072, 1024] table, then each core gathers its queries' 10 nearest neighbors'
K/V rows via indirect DMA and does the sparse attention + FFN for its rows.
"""
import os
import numpy as np

import concourse.bass as bass
import concourse.mybir as mybir
from concourse import bass_utils, bacc
from concourse.tile import TileContext

FP32 = mybir.dt.float32
BF16 = mybir.dt.bfloat16
U32 = mybir.dt.uint32
AF = mybir.ActivationFunctionType
ALU = mybir.AluOpType
AX = mybir.AxisListType

N_CORES = 8
L, GEO, SEM, RSA, D, H, HD, K = 3072, 1536, 512, 64, 256, 8, 32, 10
R = L // N_CORES           # rows per core = 384
NT = R // 128              # query tiles per core = 3
EPS = 1e-5
QSCALE = 1.0 / np.sqrt(HD)

_CACHE = {}


def build_program():
    nc = bacc.Bacc("TRN2", target_bir_lowering=False, debug=False, num_devices=N_CORES)

    # ---- I/O -------------------------------------------------------------
    def inp(name, shape, dtype=FP32):
        return nc.declare_dram_parameter(name, list(shape), dtype, isOutput=False)

    geoT = inp("geoT", [GEO, R])
    semT = inp("semT", [SEM, R])
    rsaT = inp("rsaT", [RSA, R])
    posT = inp("posT", [3, L])
    posrT = inp("posrT", [3, R])
    wg = inp("Wg", [GEO, D]); ws = inp("Ws", [SEM, D]); wr = inp("Wr", [RSA, D])
    wq = inp("Wq", [D, D]); wk = inp("Wk", [D, D])
    wv = inp("Wv", [D, D]); wo = inp("Wo", [D, D]); wt = inp("Wt", [D, D])
    wf1 = inp("Wf1", [3 * D, 2 * D]); wf2 = inp("Wf2", [2 * D, D])
    brow = inp("brow", [8, D])            # bg, bs, br, bq, bk, bt, bv, bf2 rows
    bf1_row = inp("bf1_row", [1, 2 * D])
    gb_col = inp("gb_col", [D, 8])        # cols: gg,bgn,gs,bsn,gr,brn,gt,btn
    g1b1 = inp("g1b1", [128, 4 * D])      # g1,b1,g2,b2 replicated
    gbf1 = inp("gbf1", [128, 4 * D])      # gf1,bf1n replicated (512 each)
    gbf2 = inp("gbf2", [128, 2 * D])      # gf2,bf2n replicated
    bo_rep = inp("bo_rep", [128, D])      # bo replicated
    scal = inp("scal", [128, 4])          # a_g, be_g, a_s, be_s columns

    out_fused = nc.declare_dram_parameter("fused", [R, D], FP32, isOutput=True)
    dbg_idx = nc.declare_dram_parameter("dbg_idx", [NT, 128, 16], U32, isOutput=True)
    dbg_m8 = nc.declare_dram_parameter("dbg_m8", [NT, 128, 16], FP32, isOutput=True)
    dbg_s = nc.declare_dram_parameter("dbg_s", [NT, 128, K * H], FP32, isOutput=True)
    dbg_w = nc.declare_dram_parameter("dbg_w", [NT, 128, H * K], FP32, isOutput=True)
    dbg_o = nc.declare_dram_parameter("dbg_o", [NT, 128, D], FP32, isOutput=True)
    dbg_ap = nc.declare_dram_parameter("dbg_ap", [NT, 128, D], FP32, isOutput=True)
    dbg_gk = nc.declare_dram_parameter("dbg_gk", [NT, 128, K * D], BF16, isOutput=True)
    dbg_cc = nc.declare_dram_parameter("dbg_cc", [256, 4 * D], BF16, isOutput=True)

    cc_in = nc.dram_tensor("cc_in", [R, 4 * D], BF16)
    cc_out = nc.dram_tensor("cc_out", [L, 4 * D], BF16, addr_space="Shared")

    from contextlib import ExitStack
    with TileContext(nc) as tc, ExitStack() as stack:
        pc = stack.enter_context(tc.tile_pool(name="const", bufs=1))
        pw = stack.enter_context(tc.tile_pool(name="work", bufs=1))
        pw2 = stack.enter_context(tc.tile_pool(name="work2", bufs=2))
        pg = stack.enter_context(tc.tile_pool(name="gath", bufs=2))
        ps_acc = stack.enter_context(tc.tile_pool(name="ps_acc", bufs=1, space="PSUM"))
        ps_small = stack.enter_context(tc.tile_pool(name="ps_small", bufs=1, space="PSUM"))
        ps_rep = stack.enter_context(tc.tile_pool(name="ps_rep", bufs=1, space="PSUM"))

        # ---- persistent weights/params ------------------------------------
        def load_chunks(src, rows, cols, tag):
            tiles = []
            for kc in range((rows + 127) // 128):
                p0 = kc * 128
                pn = min(128, rows - p0)
                t = pc.tile([pn, cols], FP32, tag=f"{tag}{kc}", name=f"{tag}{kc}")
                nc.sync.dma_start(out=t[:], in_=src[p0:p0 + pn, :])
                tiles.append(t)
            return tiles

        wg_sb = load_chunks(wg[:], GEO, D, "wg")
        ws_sb = load_chunks(ws[:], SEM, D, "ws")
        wr_sb = load_chunks(wr[:], RSA, D, "wr")
        wq_sb = load_chunks(wq[:], D, D, "wq")
        wk_sb = load_chunks(wk[:], D, D, "wk")
        wv_sb = load_chunks(wv[:], D, D, "wv")
        wo_sb = load_chunks(wo[:], D, D, "wo")
        wt_sb = load_chunks(wt[:], D, D, "wt")
        def load_chunks_tags(src, rows, cols, tags, names):
            tiles = []
            for kc in range((rows + 127) // 128):
                p0 = kc * 128
                pn = min(128, rows - p0)
                t = pc.tile([pn, cols], FP32, tag=tags[kc], name=names[kc])
                nc.sync.dma_start(out=t[:], in_=src[p0:p0 + pn, :])
                tiles.append(t)
            return tiles

        wf1_sb = load_chunks_tags(wf1[:], 3 * D, 2 * D,
                                  [f"wg{i}" for i in range(6)], [f"wf1_{i}" for i in range(6)])
        wf2_sb = load_chunks_tags(wf2[:], 2 * D, D,
                                  [f"wg{i}" for i in range(6, 10)], [f"wf2_{i}" for i in range(4)])

        brow_sb = []
        for bi in range(8):
            t = pc.tile([1, D], FP32, tag=f"brow{bi}", name=f"brow{bi}")
            nc.sync.dma_start(out=t[:], in_=brow[bi:bi + 1, :])
            brow_sb.append(t)
        bf1_sb = pc.tile([1, 2 * D], FP32, tag="bf1")
        nc.sync.dma_start(out=bf1_sb[:], in_=bf1_row[:])
        gb_sb = load_chunks(gb_col[:], D, 8, "gbcol")
        g1b1_sb = pc.tile([128, 4 * D], FP32, tag="g1b1")
        nc.sync.dma_start(out=g1b1_sb[:], in_=g1b1[:])
        gbf1_sb = pc.tile([128, 4 * D], FP32, tag="gbf1")
        nc.sync.dma_start(out=gbf1_sb[:], in_=gbf1[:])
        gbf2_sb = pc.tile([128, 2 * D], FP32, tag="gbf2")
        nc.sync.dma_start(out=gbf2_sb[:], in_=gbf2[:])
        bo_sb = pc.tile([128, D], FP32, tag="bo_rep")
        nc.sync.dma_start(out=bo_sb[:], in_=bo_rep[:])
        scal_sb = pc.tile([128, 4], FP32, tag="scal")
        nc.sync.dma_start(out=scal_sb[:], in_=scal[:])
        posrT_sb = pc.tile([3, R], FP32, tag="posrT")
        nc.sync.dma_start(out=posrT_sb[:], in_=posrT[:])

        ones_row = pc.tile([1, R], FP32, tag="ones_row")
        nc.vector.memset(ones_row[:], 1.0)
        ones_1 = pc.tile([1, 128], FP32, tag="ones_1p")
        nc.vector.memset(ones_1[:], 1.0)
        ones_d = pc.tile([128, 1], FP32, tag="ones_d")
        nc.vector.memset(ones_d[:], 1.0)
        ones_3 = pc.tile([3, 1], FP32, tag="ones_3")
        nc.vector.memset(ones_3[:], 1.0)
        from concourse.masks import make_identity
        ident = pc.tile([128, 128], FP32, tag="ident")
        make_identity(nc, ident[:])
        eps1 = pc.tile([1, 1], FP32, tag="eps1")
        nc.vector.memset(eps1[:], EPS)
        eps128 = pc.tile([128, 1], FP32, tag="eps128")
        nc.vector.memset(eps128[:], EPS)

        # ---- LN helpers ---------------------------------------------------
        def ln_dq(x_sb, sq_sb, g_col, b_col, out_blocks, nblk):
            """LayerNorm in [d, q]: x/sq/out are lists of [128, R] tiles."""
            dsz = nblk * 128
            mean_ps = ps_small.tile([1, R], FP32, tag="lnd_mean", name="lnd_mean")
            var_ps = ps_small.tile([1, R], FP32, tag="lnd_var", name="lnd_var")
            for b in range(nblk):
                nc.scalar.activation(out=sq_sb[b][:], in_=x_sb[b][:], func=AF.Square)
            for b in range(nblk):
                nc.tensor.matmul(mean_ps[:], ones_d[:], x_sb[b][:], start=(b == 0), stop=(b == nblk - 1))
            for b in range(nblk):
                nc.tensor.matmul(var_ps[:], ones_d[:], sq_sb[b][:], start=(b == 0), stop=(b == nblk - 1))
            negmean = pw.tile([1, R], FP32, tag="lnd_negmean", name="lnd_negmean")
            e2 = pw.tile([1, R], FP32, tag="lnd_e2", name="lnd_e2")
            msq = pw.tile([1, R], FP32, tag="lnd_msq", name="lnd_msq")
            var = pw.tile([1, R], FP32, tag="lnd_varsb", name="lnd_varsb")
            rstd = pw.tile([1, R], FP32, tag="lnd_rstd", name="lnd_rstd")
            nc.scalar.activation(out=negmean[:], in_=mean_ps[:], func=AF.Copy, scale=-1.0 / dsz)
            nc.scalar.activation(out=e2[:], in_=var_ps[:], func=AF.Copy, scale=1.0 / dsz)
            nc.scalar.activation(out=msq[:], in_=negmean[:], func=AF.Square)
            nc.vector.tensor_tensor(out=var[:], in0=e2[:], in1=msq[:], op=ALU.subtract)
            nc.scalar.activation(out=var[:], in_=var[:], func=AF.Sqrt, bias=eps1[:])
            nc.vector.reciprocal(out=rstd[:], in_=var[:])
            repm = ps_rep.tile([128, R], FP32, tag="lnd_repm", name="lnd_repm")
            reps = ps_rep.tile([128, R], FP32, tag="lnd_reps", name="lnd_reps")
            nc.tensor.matmul(repm[:], ones_1[:], negmean[:], start=True, stop=True)
            nc.tensor.matmul(reps[:], ones_1[:], rstd[:], start=True, stop=True)
            for b in range(nblk):
                u = pw.tile([128, R], FP32, tag="lnd_u", name="lnd_u")
                nc.vector.tensor_tensor(out=u[:], in0=x_sb[b][:], in1=repm[:], op=ALU.add)
                nc.vector.tensor_tensor(out=u[:], in0=u[:], in1=reps[:], op=ALU.mult)
                nc.vector.tensor_scalar(out_blocks[b][:], u[:], g_col[b], b_col[b], op0=ALU.mult, op1=ALU.add)

        def ln_qd(r_ap, g_rep, b_rep, out_ap, dsz):
            mean = pw.tile([128, 1], FP32, tag="lnq_mean", name="lnq_mean")
            nc.vector.tensor_reduce(out=mean[:], in_=r_ap, axis=AX.X, op=ALU.add)
            negmean = pw.tile([128, 1], FP32, tag="lnq_negmean", name="lnq_negmean")
            nc.scalar.activation(out=negmean[:], in_=mean[:], func=AF.Copy, scale=-1.0 / dsz)
            xm = pw.tile([128, 2 * D], FP32, tag="lnq_xm", name="lnq_xm")[:, 0:dsz]
            nc.vector.tensor_scalar(xm, r_ap, negmean[:], None, op0=ALU.add)
            sq = pw.tile([128, 2 * D], FP32, tag="lnq_sq", name="lnq_sq")[:, 0:dsz]
            nc.scalar.activation(out=sq, in_=xm, func=AF.Square)
            ssum = pw.tile([128, 1], FP32, tag="lnq_ssum", name="lnq_ssum")
            nc.vector.tensor_reduce(out=ssum[:], in_=sq, axis=AX.X, op=ALU.add)
            std = pw.tile([128, 1], FP32, tag="lnq_std", name="lnq_std")
            nc.scalar.activation(out=std[:], in_=ssum[:], func=AF.Sqrt, scale=1.0 / dsz, bias=eps128[:])
            rec = pw.tile([128, 1], FP32, tag="lnq_rec", name="lnq_rec")
            nc.vector.reciprocal(out=rec[:], in_=std[:])
            nc.vector.tensor_scalar(xm, xm, rec[:], None, op0=ALU.mult)
            nc.vector.tensor_tensor(out=xm, in0=xm, in1=g_rep, op=ALU.mult)
            nc.vector.tensor_tensor(out=out_ap, in0=xm, in1=b_rep, op=ALU.add)

        def leaky(x_ap, out_ap, dsz):
            lx = pw.tile([128, 2 * D], FP32, tag="lnq_sq", name="leaky_lx")[:, 0:dsz]
            nc.vector.tensor_scalar_mul(lx, x_ap, 0.01)
            nc.vector.tensor_max(out=out_ap, in0=x_ap, in1=lx)

        # =====================================================================
        # Stage 1: k-NN top-10  (C[i,j] = 2 p_i . p_j - |p_j|^2)
        # =====================================================================
        rhs3 = pc.tile([3, L], FP32, tag="rhs3")
        njneg = pc.tile([1, L], FP32, tag="njneg")
        for ch in range(L // 512):
            sl = slice(ch * 512, (ch + 1) * 512)
            pch = pw2.tile([3, 512], FP32, tag="pos_ch", name="pos_ch")
            nc.sync.dma_start(out=pch[:], in_=posT[:, sl])
            nc.vector.tensor_copy(rhs3[:, sl], pch[:])
            sqch = pw2.tile([3, 512], FP32, tag="sq_ch", name="sq_ch")
            nc.scalar.activation(out=sqch[:], in_=pch[:], func=AF.Square)
            nj_ps = ps_small.tile([1, 512], FP32, tag="lnd_mean", name="nj_ps")
            nc.tensor.matmul(nj_ps[:], ones_3[:], sqch[:], start=True, stop=True)
            nc.scalar.activation(out=njneg[:, sl], in_=nj_ps[:], func=AF.Copy, scale=-1.0)
        pos3 = pc.tile([3, R], FP32, tag="pos3")
        nc.vector.tensor_scalar_mul(pos3[:], posrT_sb[:], 2.0)

        idxcat = []
        for t in range(NT):
            d2n = pw2.tile([128, L], FP32, tag="d2n", name="d2n", bufs=1)
            for ch in range(L // 512):
                c_ps = ps_small.tile([128, 512], FP32, tag="c_ps", name="c_ps")
                nc.tensor.matmul(c_ps[:], pos3[:, t * 128:(t + 1) * 128],
                                 rhs3[:, ch * 512:(ch + 1) * 512], start=True, stop=False)
                nc.tensor.matmul(c_ps[:], ones_row[:, t * 128:(t + 1) * 128],
                                 njneg[:, ch * 512:(ch + 1) * 512], start=False, stop=True)
                nc.any.tensor_copy(d2n[:, ch * 512:(ch + 1) * 512], c_ps[:])
            m8 = pw.tile([128, 16], FP32, tag="m8", name="m8")
            idx = pc.tile([128, 16], U32, tag=f"idx{t}", name=f"idx{t}")
            nc.vector.max(out=m8[:, 0:8], in_=d2n[:])
            nc.vector.max_index(out=idx[:, 0:8], in_max=m8[:, 0:8], in_values=d2n[:])
            nc.vector.match_replace(out=d2n[:], in_to_replace=m8[:, 0:8], in_values=d2n[:], imm_value=-3e38)
            nc.vector.max(out=m8[:, 8:16], in_=d2n[:])
            nc.vector.max_index(out=idx[:, 8:16], in_max=m8[:, 8:16], in_values=d2n[:])
            nc.sync.dma_start(out=dbg_idx[t], in_=idx[:])
            nc.sync.dma_start(out=dbg_m8[t], in_=m8[:])
            idxcat.append(idx)

        # =====================================================================
        # Stage 2: modality projections (projT [d, q]) + LN in [d, q]
        # =====================================================================
        def modality_proj(featT_dram, nfeat, w_sb, bias_row, g_cols, b_cols, out_tiles):
            nkc = (nfeat + 127) // 128
            pss = [ps_acc.tile([128, R], FP32, tag=f"proj_ps{mb}", name=f"proj_ps{mb}")
                   for mb in range(2)]
            for kc in range(nkc):
                p0 = kc * 128
                pn = min(128, nfeat - p0)
                fch = pw2.tile([128, R], FP32, tag="featT", name="featT")
                nc.sync.dma_start(out=fch[:pn, :], in_=featT_dram[p0:p0 + pn, :])
                for mb in range(2):
                    nc.tensor.matmul(pss[mb][:], w_sb[kc][:, mb * 128:(mb + 1) * 128], fch[:pn, :],
                                     start=(kc == 0), stop=False)
            xs, sqs = [], []
            for mb in range(2):
                nc.tensor.matmul(pss[mb][:], bias_row[:, mb * 128:(mb + 1) * 128], ones_row[:],
                                 start=False, stop=True)
                x = pw.tile([128, R], FP32, tag=f"mp_x{mb}", name=f"mp_x{mb}")
                nc.any.tensor_copy(x[:], pss[mb][:])
                sq = pw.tile([128, R], FP32, tag=f"mp_sq{mb}", name=f"mp_sq{mb}")
                xs.append(x); sqs.append(sq)
            ln_dq(xs, sqs, g_cols, b_cols, out_tiles, 2)

        geoPT = [pc.tile([128, R], FP32, tag=f"geoPT{b}", name=f"geoPT{b}") for b in range(2)]
        semPT = [pc.tile([128, R], FP32, tag=f"semPT{b}", name=f"semPT{b}") for b in range(2)]
        rsaPT = [pw.tile([128, R], FP32, tag=f"rsaPT{b}", name=f"rsaPT{b}") for b in range(2)]
        modality_proj(geoT[:], GEO, wg_sb, brow_sb[0][:],
                      [gb_sb[b][:, 0:1] for b in range(2)], [gb_sb[b][:, 1:2] for b in range(2)], geoPT)
        modality_proj(semT[:], SEM, ws_sb, brow_sb[1][:],
                      [gb_sb[b][:, 2:3] for b in range(2)], [gb_sb[b][:, 3:4] for b in range(2)], semPT)
        modality_proj(rsaT[:], RSA, wr_sb, brow_sb[2][:],
                      [gb_sb[b][:, 4:5] for b in range(2)], [gb_sb[b][:, 5:6] for b in range(2)], rsaPT)

        # =====================================================================
        # Stage 3: WvWo fold, Q/K/V projections, KV table [k1|k2|v1|v2], AllGather
        # =====================================================================
        def rows_mm(srcPT, w_chunks, bias_row, t):
            o_ps = ps_acc.tile([128, D], FP32, tag="qkv_ps", name="qkv_ps")
            for kc in range(2):
                nc.tensor.matmul(o_ps[:], srcPT[kc][:, t * 128:(t + 1) * 128], w_chunks[kc][:],
                                 start=(kc == 0), stop=False)
            nc.tensor.matmul(o_ps[:], ones_1[:], bias_row, start=False, stop=True)
            return o_ps

        q1t, q2t = [], []
        for t in range(NT):
            ps = rows_mm(geoPT, wq_sb, brow_sb[3][:], t)
            q1 = pc.tile([128, D], BF16, tag=f"q1_{t}", name=f"q1_{t}")
            nc.scalar.activation(out=q1[:], in_=ps[:], func=AF.Copy, scale=QSCALE)
            q1t.append(q1)
            ps = rows_mm(semPT, wq_sb, brow_sb[3][:], t)
            q2 = pc.tile([128, D], BF16, tag=f"q2_{t}", name=f"q2_{t}")
            nc.scalar.activation(out=q2[:], in_=ps[:], func=AF.Copy, scale=QSCALE)
            q2t.append(q2)

            kv = pw2.tile([128, 4 * D], BF16, tag="kv", name="kv")
            ps = rows_mm(semPT, wk_sb, brow_sb[4][:], t)
            nc.any.tensor_copy(kv[:, 0:D], ps[:])
            ps = rows_mm(geoPT, wk_sb, brow_sb[4][:], t)
            nc.any.tensor_copy(kv[:, D:2 * D], ps[:])
            ps = rows_mm(semPT, wv_sb, brow_sb[6][:], t)
            nc.any.tensor_copy(kv[:, 2 * D:3 * D], ps[:])
            ps = rows_mm(geoPT, wv_sb, brow_sb[6][:], t)
            nc.any.tensor_copy(kv[:, 3 * D:4 * D], ps[:])
            nc.sync.dma_start(out=cc_in[t * 128:(t + 1) * 128, :], in_=kv[:])

        nc.gpsimd.collective_compute(
            "AllGather", ALU.bypass,
            replica_groups=[list(range(N_CORES))],
            ins=[cc_in[:]], outs=[cc_out[:]],
        )
        ccdump = pw2.tile([128, 4 * D], BF16, tag="kv", name="ccdump")
        nc.sync.dma_start(out=ccdump[:], in_=cc_out[0:128, :])
        nc.sync.dma_start(out=dbg_cc[0:128], in_=ccdump[:])
        ccdump2 = pw2.tile([128, 4 * D], BF16, tag="kv", name="ccdump2")
        nc.sync.dma_start(out=ccdump2[:], in_=cc_out[384:512, :])
        nc.sync.dma_start(out=dbg_cc[128:256], in_=ccdump2[:])

        # =====================================================================
        # Stage 4: rsa transform (in [d, q]) -> combinedT rows 512..767
        # =====================================================================
        combT = [pc.tile([128, R], FP32, tag=f"combT{i}", name=f"combT{i}") for i in range(6)]
        r_xs, r_sqs, r_outs = [], [], []
        for mb in range(2):
            p_ps = ps_acc.tile([128, R], FP32, tag=f"proj_ps{mb}", name=f"rsa2_ps{mb}")
            for kc in range(2):
                nc.tensor.matmul(p_ps[:], wt_sb[kc][:, mb * 128:(mb + 1) * 128], rsaPT[kc][:],
                                 start=(kc == 0), stop=False)
            nc.tensor.matmul(p_ps[:], brow_sb[5][:, mb * 128:(mb + 1) * 128], ones_row[:],
                             start=False, stop=True)
            x = pw.tile([128, R], FP32, tag=f"mp_x{mb}", name=f"rsa2_x{mb}")
            nc.any.tensor_copy(x[:], p_ps[:])
            sq = pw.tile([128, R], FP32, tag=f"mp_sq{mb}", name=f"rsa2_sq{mb}")
            o = pw.tile([128, R], FP32, tag=f"rsa2_ln{mb}", name=f"rsa2_ln{mb}")
            r_xs.append(x); r_sqs.append(sq); r_outs.append(o)
        ln_dq(r_xs, r_sqs, [gb_sb[b][:, 6:7] for b in range(2)],
              [gb_sb[b][:, 7:8] for b in range(2)], r_outs, 2)
        for mb in range(2):
            lx = pw.tile([128, R], FP32, tag="rsa_leak", name="rsa_leak")
            nc.vector.tensor_scalar_mul(lx[:], r_outs[mb][:], 0.01)
            nc.vector.tensor_max(out=combT[4 + mb][:], in0=r_outs[mb][:], in1=lx[:])

        # =====================================================================
        # Stage 5+6: per-tile attention, residual LNs, FFN
        # =====================================================================
        hT = [pc.tile([128, R], FP32, tag=f"hT{i}", name=f"hT{i}") for i in range(4)]

        def transpose_to(dst_ap, src_ap):
            tp = ps_rep.tile([128, 128], FP32, tag="lnd_reps", name="transp")
            nc.tensor.transpose(out=tp[:], in_=src_ap, identity=ident[:])
            nc.any.tensor_copy(dst_ap, tp[:])

        for t in range(NT):
            g_sb = pg.tile([128, K, 4 * D], BF16, tag="g_sb", name="g_sb", bufs=1)
            for n in range(K):
                nc.gpsimd.indirect_dma_start(
                    out=g_sb[:, n, :], out_offset=None, in_=cc_out[:],
                    in_offset=bass.IndirectOffsetOnAxis(ap=idxcat[t][:, n:n + 1], axis=0),
                )

            def attn(qtile, off_k, off_v, dbg=False):
                prod = pw2.tile([128, K * D], BF16, tag="prod", name="prod")
                qb = qtile[:].rearrange("p (r e) -> p r e", r=1)
                nc.vector.tensor_tensor(out=prod[:], in0=g_sb[:, :, off_k:off_k + D],
                                        in1=qb.to_broadcast([128, K, D]), op=ALU.mult)
                s = pw.tile([128, K * H], FP32, tag="s_nh", name="s_nh")  # [n][h]
                nc.vector.tensor_reduce(out=s[:], in_=prod[:].rearrange("p (g d) -> p g d", d=HD),
                                        axis=AX.X, op=ALU.add)
                if dbg:
                    nc.sync.dma_start(out=dbg_s[t], in_=s[:])
                    nc.sync.dma_start(out=dbg_gk[t], in_=g_sb[:, :, off_k:off_k + D])
                mx = pw.tile([128, H], FP32, tag="s_mx", name="s_mx")
                s_hn = s[:].rearrange("p (n h) -> p h n", h=H)
                nc.vector.tensor_reduce(out=mx[:], in_=s_hn, axis=AX.X, op=ALU.max)
                z = pw.tile([128, H * K], FP32, tag="s_z", name="s_z")   # [h][n]
                mxb = mx[:].rearrange("p (h r) -> p h r", r=1)
                nc.vector.tensor_tensor(out=z[:], in0=s_hn, in1=mxb.to_broadcast([128, H, K]), op=ALU.subtract)
                w = pw.tile([128, H * K], FP32, tag="s_w", name="s_w")
                nc.scalar.activation(out=w[:], in_=z[:], func=AF.Exp)
                sm = pw.tile([128, H], FP32, tag="s_sm", name="s_sm")
                nc.vector.tensor_reduce(out=sm[:], in_=w[:].rearrange("p (h n) -> p h n", h=H),
                                        axis=AX.X, op=ALU.add)
                rec = pw.tile([128, H], FP32, tag="s_rec", name="s_rec")
                nc.vector.reciprocal(out=rec[:], in_=sm[:])
                recb = rec[:].rearrange("p (h r) -> p h r", r=1)
                nc.vector.tensor_tensor(out=w[:], in0=w[:], in1=recb.to_broadcast([128, H, K]), op=ALU.mult)
                if dbg:
                    nc.sync.dma_start(out=dbg_w[t], in_=w[:])
                prod2 = pw2.tile([128, D * K], BF16, tag="prod", name="prod2")
                vview = g_sb[:, :, off_v:off_v + D].rearrange("p n (h d) -> p n h d", d=HD)
                wview = w[:].rearrange("p (h n o) -> p n h o", h=H, o=1)
                p2view = prod2[:].rearrange("p (h d n) -> p n h d", h=H, d=HD)
                nc.vector.tensor_tensor(out=p2view, in0=vview, in1=wview.to_broadcast([128, K, H, HD]), op=ALU.mult)
                o = pw.tile([128, D], FP32, tag="attn_o", name="attn_o")
                nc.vector.tensor_reduce(out=o[:], in_=prod2[:].rearrange("p (g n) -> p g n", n=K),
                                        axis=AX.X, op=ALU.add)
                oT = pw.tile([128, D], FP32, tag="oT", name="oT")
                for mb in range(2):
                    transpose_to(oT[:, mb * 128:(mb + 1) * 128], o[:, mb * 128:(mb + 1) * 128])
                op_ps = ps_acc.tile([128, D], FP32, tag="qkv_ps", name="op_ps")
                for kc in range(2):
                    nc.tensor.matmul(op_ps[:], oT[:, kc * 128:(kc + 1) * 128], wo_sb[kc][:],
                                     start=(kc == 0), stop=(kc == 1))
                ap = pw.tile([128, D], FP32, tag="attn_p", name="attn_p")
                nc.any.tensor_copy(ap[:], op_ps[:])
                if dbg:
                    nc.sync.dma_start(out=dbg_o[t], in_=o[:])
                    nc.sync.dma_start(out=dbg_ap[t], in_=ap[:])
                return ap

            def residual_ln(attn_o, p_qd, a_col, be_col, g_rep, b_rep, out_ap):
                t1 = pw.tile([128, D], FP32, tag="res_t1", name="res_t1")
                nc.vector.tensor_tensor(out=t1[:], in0=attn_o[:], in1=bo_sb[:], op=ALU.add)
                nc.vector.tensor_scalar(t1[:], t1[:], be_col, None, op0=ALU.mult)
                t2 = pw.tile([128, D], FP32, tag="res_t2", name="res_t2")
                nc.vector.tensor_scalar(t2[:], p_qd, a_col, None, op0=ALU.mult)
                nc.vector.tensor_tensor(out=t1[:], in0=t1[:], in1=t2[:], op=ALU.add)
                ln_qd(t1[:], g_rep, b_rep, out_ap, D)

            attn1 = attn(q1t[t], 0, 2 * D, dbg=True)
            geo_qd = pw.tile([128, D], FP32, tag="geo_qd", name="geo_qd")
            for mb in range(2):
                transpose_to(geo_qd[:, mb * 128:(mb + 1) * 128], geoPT[mb][:, t * 128:(t + 1) * 128])
            geo_out = pw.tile([128, D], FP32, tag="geo_out", name="geo_out")
            residual_ln(attn1, geo_qd[:], scal_sb[:, 0:1], scal_sb[:, 1:2],
                        g1b1_sb[:, 0:D], g1b1_sb[:, D:2 * D], geo_out[:])
            for mb in range(2):
                transpose_to(combT[mb][:, t * 128:(t + 1) * 128], geo_out[:, mb * 128:(mb + 1) * 128])

            attn2 = attn(q2t[t], D, 3 * D)
            sem_qd = pw.tile([128, D], FP32, tag="sem_qd", name="sem_qd")
            for mb in range(2):
                transpose_to(sem_qd[:, mb * 128:(mb + 1) * 128], semPT[mb][:, t * 128:(t + 1) * 128])
            sem_out = pw.tile([128, D], FP32, tag="sem_out", name="sem_out")
            residual_ln(attn2, sem_qd[:], scal_sb[:, 2:3], scal_sb[:, 3:4],
                        g1b1_sb[:, 2 * D:3 * D], g1b1_sb[:, 3 * D:4 * D], sem_out[:])
            for mb in range(2):
                transpose_to(combT[2 + mb][:, t * 128:(t + 1) * 128], sem_out[:, mb * 128:(mb + 1) * 128])

            # ---- FFN layer 1 ----
            f1_ps = ps_small.tile([128, 2 * D], FP32, tag="c_ps", name="f1_ps")
            for kc in range(6):
                nc.tensor.matmul(f1_ps[:], combT[kc][:, t * 128:(t + 1) * 128], wf1_sb[kc][:],
                                 start=(kc == 0), stop=False)
            nc.tensor.matmul(f1_ps[:], ones_1[:], bf1_sb[:], start=False, stop=True)
            f1x = pw.tile([128, 2 * D], FP32, tag="f1x", name="f1x")
            nc.any.tensor_copy(f1x[:], f1_ps[:])
            f1ln = pw.tile([128, 2 * D], FP32, tag="f1x", name="f1ln")
            ln_qd(f1x[:], gbf1_sb[:, 0:2 * D], gbf1_sb[:, 2 * D:4 * D], f1ln[:], 2 * D)
            h_t = pw.tile([128, 2 * D], FP32, tag="h_t", name="h_t")
            leaky(f1ln[:], h_t[:], 2 * D)
            for mb in range(4):
                transpose_to(hT[mb][:, t * 128:(t + 1) * 128], h_t[:, mb * 128:(mb + 1) * 128])

            # ---- FFN layer 2 ----
            f2_ps = ps_acc.tile([128, D], FP32, tag="qkv_ps", name="f2_ps")
            for kc in range(4):
                nc.tensor.matmul(f2_ps[:], hT[kc][:, t * 128:(t + 1) * 128], wf2_sb[kc][:],
                                 start=(kc == 0), stop=False)
            nc.tensor.matmul(f2_ps[:], ones_1[:], brow_sb[7][:], start=False, stop=True)
            f2x = pw.tile([128, D], FP32, tag="f2x", name="f2x")
            nc.any.tensor_copy(f2x[:], f2_ps[:])
            f2ln = pw.tile([128, D], FP32, tag="f2x", name="f2ln")
            ln_qd(f2x[:], gbf2_sb[:, 0:D], gbf2_sb[:, D:2 * D], f2ln[:], D)
            fout = pw.tile([128, D], FP32, tag="res_t1", name="fout")
            leaky(f2ln[:], fout[:], D)
            nc.sync.dma_start(out=out_fused[t * 128:(t + 1) * 128, :], in_=fout[:])

    nc.compile()
    return nc


def make_in_maps(geo_feat, sem_feat, rsa_feat, pos, params):
    p = params
    f32 = np.float32
    def T(x):
        return np.ascontiguousarray(np.asarray(x, dtype=f32).T)

    posT = T(pos)
    brow = np.stack([np.asarray(p[k], f32) for k in
                     ("bg", "bs", "br", "bq", "bk", "bt", "bv", "bf2")])
    gb_col = np.stack([np.asarray(p[k], f32) for k in
                       ("gg", "bgn", "gs", "bsn", "gr", "brn", "gt", "btn")], axis=1)
    g1b1 = np.concatenate([np.tile(np.asarray(p[k], f32)[None, :], (128, 1))
                           for k in ("g1", "b1", "g2", "b2")], axis=1)
    gbf1 = np.concatenate([np.tile(np.asarray(p[k], f32)[None, :], (128, 1))
                           for k in ("gf1", "bf1n")], axis=1)
    gbf2 = np.concatenate([np.tile(np.asarray(p[k], f32)[None, :], (128, 1))
                           for k in ("gf2", "bf2n")], axis=1)
    bo_rep = np.tile(np.asarray(p["bo"], f32)[None, :], (128, 1))
    scal = np.tile(np.array([p["a_g"], p["be_g"], p["a_s"], p["be_s"]], f32)[None, :], (128, 1))

    shared = {
        "posT": posT,
        "Wg": np.asarray(p["Wg"], f32), "Ws": np.asarray(p["Ws"], f32), "Wr": np.asarray(p["Wr"], f32),
        "Wq": np.asarray(p["Wq"], f32), "Wk": np.asarray(p["Wk"], f32),
        "Wv": np.asarray(p["Wv"], f32), "Wo": np.asarray(p["Wo"], f32), "Wt": np.asarray(p["Wt"], f32),
        "Wf1": np.asarray(p["Wf1"], f32), "Wf2": np.asarray(p["Wf2"], f32),
        "brow": np.ascontiguousarray(brow), "bf1_row": np.asarray(p["bf1"], f32)[None, :],
        "gb_col": np.ascontiguousarray(gb_col),
        "g1b1": g1b1, "gbf1": gbf1, "gbf2": gbf2, "bo_rep": bo_rep, "scal": scal,
    }
    in_maps = []
    for c in range(N_CORES):
        r0 = c * R
        m = dict(shared)
        m["geoT"] = T(np.asarray(geo_feat)[r0:r0 + R])
        m["semT"] = T(np.asarray(sem_feat)[r0:r0 + R])
        m["rsaT"] = T(np.asarray(rsa_feat)[r0:r0 + R])
        m["posrT"] = np.ascontiguousarray(posT[:, r0:r0 + R])
        in_maps.append(m)
    return in_maps


def kernel(geo_feat, sem_feat, rsa_feat, pos, params):
    if "nc" not in _CACHE:
        _CACHE["nc"] = build_program()
    nc = _CACHE["nc"]
    in_maps = make_in_maps(geo_feat, sem_feat, rsa_feat, pos, params)

    if os.environ.get("KERNEL_SIM"):
        from concourse.bass_interp import MultiCoreSim
        sim = MultiCoreSim(nc, num_cores=N_CORES, trace=False)
        for c in range(N_CORES):
            core = sim.cores[c]
            for k, v in in_maps[c].items():
                core.tensor(k)[:] = v
        sim.simulate(check_with_hw=False)
        outs = [np.array(sim.cores[c].tensor("fused")) for c in range(N_CORES)]
    else:
        res = bass_utils.run_bass_kernel_spmd(nc, in_maps, list(range(N_CORES)))
        outs = [np.asarray(res.results[c]["fused"]) for c in range(N_CORES)]
    return np.concatenate(outs, axis=0)


# revision 20
# speedup vs baseline: 2.0805x; 2.0805x over previous
"""Trainium2 Bass kernel for nn_LocalCrossModalAttention (sparse k-NN cross attention).

Sharding: rows (queries) L=3072 split across 8 cores (384 each). K/V projections
(with Wo folded into V) are computed row-parallel, AllGathered as a bf16
[3072, 1024] table, then each core gathers its queries' 10 nearest neighbors'
K/V rows via indirect DMA and does the sparse attention + FFN for its rows.
"""
import os
import numpy as np

import concourse.bass as bass
import concourse.mybir as mybir
from concourse import bass_utils, bacc
from concourse.tile import TileContext

FP32 = mybir.dt.float32
BF16 = mybir.dt.bfloat16
U32 = mybir.dt.uint32
AF = mybir.ActivationFunctionType
ALU = mybir.AluOpType
AX = mybir.AxisListType

N_CORES = 8
L, GEO, SEM, RSA, D, H, HD, K = 3072, 1536, 512, 64, 256, 8, 32, 10
R = L // N_CORES           # rows per core = 384
NT = R // 128              # query tiles per core = 3
EPS = 1e-5
QSCALE = 1.0 / np.sqrt(HD)

_CACHE = {}


def build_program():
    nc = bacc.Bacc("TRN2", target_bir_lowering=False, debug=False, num_devices=N_CORES)

    # ---- I/O -------------------------------------------------------------
    def inp(name, shape, dtype=FP32):
        return nc.declare_dram_parameter(name, list(shape), dtype, isOutput=False)

    geoT = inp("geoT", [GEO, R])
    semT = inp("semT", [SEM, R])
    rsaT = inp("rsaT", [RSA, R])
    posT = inp("posT", [3, L])
    posrT = inp("posrT", [3, R])
    wg = inp("Wg", [GEO, D]); ws = inp("Ws", [SEM, D]); wr = inp("Wr", [RSA, D])
    wq = inp("Wq", [D, D]); wk = inp("Wk", [D, D])
    wv = inp("Wv", [D, D]); wo = inp("Wo", [D, D]); wt = inp("Wt", [D, D])
    wf1 = inp("Wf1", [3 * D, 2 * D]); wf2 = inp("Wf2", [2 * D, D])
    brow = inp("brow", [8, D])            # bg, bs, br, bq, bk, bt, bv, bf2 rows
    bf1_row = inp("bf1_row", [1, 2 * D])
    gb_col = inp("gb_col", [D, 8])        # cols: gg,bgn,gs,bsn,gr,brn,gt,btn
    g1b1 = inp("g1b1", [128, 4 * D])      # g1,b1,g2,b2 replicated
    gbf1 = inp("gbf1", [128, 4 * D])      # gf1,bf1n replicated (512 each)
    gbf2 = inp("gbf2", [128, 2 * D])      # gf2,bf2n replicated
    bo_rep = inp("bo_rep", [128, D])      # bo replicated
    scal = inp("scal", [128, 4])          # a_g, be_g, a_s, be_s columns

    out_fused = nc.declare_dram_parameter("fused", [R, D], FP32, isOutput=True)

    cc_in = nc.dram_tensor("cc_in", [R, 4 * D], BF16)
    cc_out = nc.dram_tensor("cc_out", [L, 4 * D], BF16, addr_space="Shared")

    from contextlib import ExitStack
    with TileContext(nc) as tc, ExitStack() as stack:
        pc = stack.enter_context(tc.tile_pool(name="const", bufs=1))
        pw = stack.enter_context(tc.tile_pool(name="work", bufs=1))
        pw2 = stack.enter_context(tc.tile_pool(name="work2", bufs=2))
        pg = stack.enter_context(tc.tile_pool(name="gath", bufs=2))
        ps_acc = stack.enter_context(tc.tile_pool(name="ps_acc", bufs=1, space="PSUM"))
        ps_small = stack.enter_context(tc.tile_pool(name="ps_small", bufs=1, space="PSUM"))
        ps_rep = stack.enter_context(tc.tile_pool(name="ps_rep", bufs=1, space="PSUM"))

        # ---- persistent weights/params ------------------------------------
        def load_chunks(src, rows, cols, tag):
            tiles = []
            for kc in range((rows + 127) // 128):
                p0 = kc * 128
                pn = min(128, rows - p0)
                t = pc.tile([pn, cols], FP32, tag=f"{tag}{kc}", name=f"{tag}{kc}")
                nc.sync.dma_start(out=t[:], in_=src[p0:p0 + pn, :])
                tiles.append(t)
            return tiles

        wg_sb = load_chunks(wg[:], GEO, D, "wg")
        ws_sb = load_chunks(ws[:], SEM, D, "ws")
        wr_sb = load_chunks(wr[:], RSA, D, "wr")
        wq_sb = load_chunks(wq[:], D, D, "wq")
        wk_sb = load_chunks(wk[:], D, D, "wk")
        wv_sb = load_chunks(wv[:], D, D, "wv")
        wo_sb = load_chunks(wo[:], D, D, "wo")
        wt_sb = load_chunks(wt[:], D, D, "wt")
        def load_chunks_tags(src, rows, cols, tags, names):
            tiles = []
            for kc in range((rows + 127) // 128):
                p0 = kc * 128
                pn = min(128, rows - p0)
                t = pc.tile([pn, cols], FP32, tag=tags[kc], name=names[kc])
                nc.sync.dma_start(out=t[:], in_=src[p0:p0 + pn, :])
                tiles.append(t)
            return tiles

        wf1_sb = load_chunks_tags(wf1[:], 3 * D, 2 * D,
                                  [f"wg{i}" for i in range(6)], [f"wf1_{i}" for i in range(6)])
        wf2_sb = load_chunks_tags(wf2[:], 2 * D, D,
                                  [f"wg{i}" for i in range(6, 10)], [f"wf2_{i}" for i in range(4)])

        brow_sb = []
        for bi in range(8):
            t = pc.tile([1, D], FP32, tag=f"brow{bi}", name=f"brow{bi}")
            nc.sync.dma_start(out=t[:], in_=brow[bi:bi + 1, :])
            brow_sb.append(t)
        bf1_sb = pc.tile([1, 2 * D], FP32, tag="bf1")
        nc.sync.dma_start(out=bf1_sb[:], in_=bf1_row[:])
        gb_sb = load_chunks(gb_col[:], D, 8, "gbcol")
        g1b1_sb = pc.tile([128, 4 * D], FP32, tag="g1b1")
        nc.sync.dma_start(out=g1b1_sb[:], in_=g1b1[:])
        gbf1_sb = pc.tile([128, 4 * D], FP32, tag="gbf1")
        nc.sync.dma_start(out=gbf1_sb[:], in_=gbf1[:])
        gbf2_sb = pc.tile([128, 2 * D], FP32, tag="gbf2")
        nc.sync.dma_start(out=gbf2_sb[:], in_=gbf2[:])
        bo_sb = pc.tile([128, D], FP32, tag="bo_rep")
        nc.sync.dma_start(out=bo_sb[:], in_=bo_rep[:])
        scal_sb = pc.tile([128, 4], FP32, tag="scal")
        nc.sync.dma_start(out=scal_sb[:], in_=scal[:])
        posrT_sb = pc.tile([3, R], FP32, tag="posrT")
        nc.sync.dma_start(out=posrT_sb[:], in_=posrT[:])

        ones_row = pc.tile([1, R], FP32, tag="ones_row")
        nc.vector.memset(ones_row[:], 1.0)
        ones_1 = pc.tile([1, 128], FP32, tag="ones_1p")
        nc.vector.memset(ones_1[:], 1.0)
        ones_d = pc.tile([128, 1], FP32, tag="ones_d")
        nc.vector.memset(ones_d[:], 1.0)
        ones_3 = pc.tile([3, 1], FP32, tag="ones_3")
        nc.vector.memset(ones_3[:], 1.0)
        from concourse.masks import make_identity
        ident = pc.tile([128, 128], FP32, tag="ident")
        make_identity(nc, ident[:])
        eps1 = pc.tile([1, 1], FP32, tag="eps1")
        nc.vector.memset(eps1[:], EPS)
        eps128 = pc.tile([128, 1], FP32, tag="eps128")
        nc.vector.memset(eps128[:], EPS)

        # ---- LN helpers ---------------------------------------------------
        def ln_dq(x_sb, sq_sb, g_col, b_col, out_blocks, nblk):
            """LayerNorm in [d, q]: x/sq/out are lists of [128, R] tiles."""
            dsz = nblk * 128
            mean_ps = ps_small.tile([1, R], FP32, tag="lnd_mean", name="lnd_mean")
            var_ps = ps_small.tile([1, R], FP32, tag="lnd_var", name="lnd_var")
            for b in range(nblk):
                nc.scalar.activation(out=sq_sb[b][:], in_=x_sb[b][:], func=AF.Square)
            for b in range(nblk):
                nc.tensor.matmul(mean_ps[:], ones_d[:], x_sb[b][:], start=(b == 0), stop=(b == nblk - 1))
            for b in range(nblk):
                nc.tensor.matmul(var_ps[:], ones_d[:], sq_sb[b][:], start=(b == 0), stop=(b == nblk - 1))
            negmean = pw.tile([1, R], FP32, tag="lnd_negmean", name="lnd_negmean")
            e2 = pw.tile([1, R], FP32, tag="lnd_e2", name="lnd_e2")
            msq = pw.tile([1, R], FP32, tag="lnd_msq", name="lnd_msq")
            var = pw.tile([1, R], FP32, tag="lnd_varsb", name="lnd_varsb")
            rstd = pw.tile([1, R], FP32, tag="lnd_rstd", name="lnd_rstd")
            nc.scalar.activation(out=negmean[:], in_=mean_ps[:], func=AF.Copy, scale=-1.0 / dsz)
            nc.scalar.activation(out=e2[:], in_=var_ps[:], func=AF.Copy, scale=1.0 / dsz)
            nc.scalar.activation(out=msq[:], in_=negmean[:], func=AF.Square)
            nc.vector.tensor_tensor(out=var[:], in0=e2[:], in1=msq[:], op=ALU.subtract)
            nc.scalar.activation(out=var[:], in_=var[:], func=AF.Sqrt, bias=eps1[:])
            nc.vector.reciprocal(out=rstd[:], in_=var[:])
            repm = ps_rep.tile([128, R], FP32, tag="lnd_repm", name="lnd_repm")
            reps = ps_rep.tile([128, R], FP32, tag="lnd_reps", name="lnd_reps")
            nc.tensor.matmul(repm[:], ones_1[:], negmean[:], start=True, stop=True)
            nc.tensor.matmul(reps[:], ones_1[:], rstd[:], start=True, stop=True)
            for b in range(nblk):
                u = pw.tile([128, R], FP32, tag="lnd_u", name="lnd_u")
                nc.vector.tensor_tensor(out=u[:], in0=x_sb[b][:], in1=repm[:], op=ALU.add)
                nc.vector.tensor_tensor(out=u[:], in0=u[:], in1=reps[:], op=ALU.mult)
                nc.vector.tensor_scalar(out_blocks[b][:], u[:], g_col[b], b_col[b], op0=ALU.mult, op1=ALU.add)

        def ln_qd(r_ap, g_rep, b_rep, out_ap, dsz):
            mean = pw.tile([128, 1], FP32, tag="lnq_mean", name="lnq_mean")
            nc.vector.tensor_reduce(out=mean[:], in_=r_ap, axis=AX.X, op=ALU.add)
            negmean = pw.tile([128, 1], FP32, tag="lnq_negmean", name="lnq_negmean")
            nc.scalar.activation(out=negmean[:], in_=mean[:], func=AF.Copy, scale=-1.0 / dsz)
            xm = pw.tile([128, 2 * D], FP32, tag="lnq_xm", name="lnq_xm")[:, 0:dsz]
            nc.vector.tensor_scalar(xm, r_ap, negmean[:], None, op0=ALU.add)
            sq = pw.tile([128, 2 * D], FP32, tag="lnq_sq", name="lnq_sq")[:, 0:dsz]
            nc.scalar.activation(out=sq, in_=xm, func=AF.Square)
            ssum = pw.tile([128, 1], FP32, tag="lnq_ssum", name="lnq_ssum")
            nc.vector.tensor_reduce(out=ssum[:], in_=sq, axis=AX.X, op=ALU.add)
            std = pw.tile([128, 1], FP32, tag="lnq_std", name="lnq_std")
            nc.scalar.activation(out=std[:], in_=ssum[:], func=AF.Sqrt, scale=1.0 / dsz, bias=eps128[:])
            rec = pw.tile([128, 1], FP32, tag="lnq_rec", name="lnq_rec")
            nc.vector.reciprocal(out=rec[:], in_=std[:])
            nc.vector.tensor_scalar(xm, xm, rec[:], None, op0=ALU.mult)
            nc.vector.tensor_tensor(out=xm, in0=xm, in1=g_rep, op=ALU.mult)
            nc.vector.tensor_tensor(out=out_ap, in0=xm, in1=b_rep, op=ALU.add)

        def leaky(x_ap, out_ap, dsz):
            lx = pw.tile([128, 2 * D], FP32, tag="lnq_sq", name="leaky_lx")[:, 0:dsz]
            nc.vector.tensor_scalar_mul(lx, x_ap, 0.01)
            nc.vector.tensor_max(out=out_ap, in0=x_ap, in1=lx)

        # =====================================================================
        # Stage 1: k-NN top-10  (C[i,j] = 2 p_i . p_j - |p_j|^2)
        # =====================================================================
        rhs3 = pc.tile([3, L], FP32, tag="rhs3")
        njneg = pc.tile([1, L], FP32, tag="njneg")
        for ch in range(L // 512):
            sl = slice(ch * 512, (ch + 1) * 512)
            pch = pw2.tile([3, 512], FP32, tag="pos_ch", name="pos_ch")
            nc.sync.dma_start(out=pch[:], in_=posT[:, sl])
            nc.vector.tensor_copy(rhs3[:, sl], pch[:])
            sqch = pw2.tile([3, 512], FP32, tag="sq_ch", name="sq_ch")
            nc.scalar.activation(out=sqch[:], in_=pch[:], func=AF.Square)
            nj_ps = ps_small.tile([1, 512], FP32, tag="lnd_mean", name="nj_ps")
            nc.tensor.matmul(nj_ps[:], ones_3[:], sqch[:], start=True, stop=True)
            nc.scalar.activation(out=njneg[:, sl], in_=nj_ps[:], func=AF.Copy, scale=-1.0)
        pos3 = pc.tile([3, R], FP32, tag="pos3")
        nc.vector.tensor_scalar_mul(pos3[:], posrT_sb[:], 2.0)

        idxcat = []
        for t in range(NT):
            d2n = pw2.tile([128, L], FP32, tag="d2n", name="d2n", bufs=1)
            for ch in range(L // 512):
                c_ps = ps_small.tile([128, 512], FP32, tag="c_ps", name="c_ps")
                nc.tensor.matmul(c_ps[:], pos3[:, t * 128:(t + 1) * 128],
                                 rhs3[:, ch * 512:(ch + 1) * 512], start=True, stop=False)
                nc.tensor.matmul(c_ps[:], ones_row[:, t * 128:(t + 1) * 128],
                                 njneg[:, ch * 512:(ch + 1) * 512], start=False, stop=True)
                nc.any.tensor_copy(d2n[:, ch * 512:(ch + 1) * 512], c_ps[:])
            m8 = pw.tile([128, 16], FP32, tag="m8", name="m8")
            idx = pc.tile([128, 16], U32, tag=f"idx{t}", name=f"idx{t}")
            nc.vector.max(out=m8[:, 0:8], in_=d2n[:])
            nc.vector.max_index(out=idx[:, 0:8], in_max=m8[:, 0:8], in_values=d2n[:])
            nc.vector.match_replace(out=d2n[:], in_to_replace=m8[:, 0:8], in_values=d2n[:], imm_value=-3e38)
            nc.vector.max(out=m8[:, 8:16], in_=d2n[:])
            nc.vector.max_index(out=idx[:, 8:16], in_max=m8[:, 8:16], in_values=d2n[:])
            idxcat.append(idx)

        # =====================================================================
        # Stage 2: modality projections (projT [d, q]) + LN in [d, q]
        # =====================================================================
        def modality_proj(featT_dram, nfeat, w_sb, bias_row, g_cols, b_cols, out_tiles):
            nkc = (nfeat + 127) // 128
            pss = [ps_acc.tile([128, R], FP32, tag=f"proj_ps{mb}", name=f"proj_ps{mb}")
                   for mb in range(2)]
            for kc in range(nkc):
                p0 = kc * 128
                pn = min(128, nfeat - p0)
                fch = pw2.tile([128, R], FP32, tag="featT", name="featT")
                nc.sync.dma_start(out=fch[:pn, :], in_=featT_dram[p0:p0 + pn, :])
                for mb in range(2):
                    nc.tensor.matmul(pss[mb][:], w_sb[kc][:, mb * 128:(mb + 1) * 128], fch[:pn, :],
                                     start=(kc == 0), stop=False)
            xs, sqs = [], []
            for mb in range(2):
                nc.tensor.matmul(pss[mb][:], bias_row[:, mb * 128:(mb + 1) * 128], ones_row[:],
                                 start=False, stop=True)
                x = pw.tile([128, R], FP32, tag=f"mp_x{mb}", name=f"mp_x{mb}")
                nc.any.tensor_copy(x[:], pss[mb][:])
                sq = pw.tile([128, R], FP32, tag=f"mp_sq{mb}", name=f"mp_sq{mb}")
                xs.append(x); sqs.append(sq)
            ln_dq(xs, sqs, g_cols, b_cols, out_tiles, 2)

        geoPT = [pc.tile([128, R], FP32, tag=f"geoPT{b}", name=f"geoPT{b}") for b in range(2)]
        semPT = [pc.tile([128, R], FP32, tag=f"semPT{b}", name=f"semPT{b}") for b in range(2)]
        rsaPT = [pw.tile([128, R], FP32, tag=f"rsaPT{b}", name=f"rsaPT{b}") for b in range(2)]
        modality_proj(geoT[:], GEO, wg_sb, brow_sb[0][:],
                      [gb_sb[b][:, 0:1] for b in range(2)], [gb_sb[b][:, 1:2] for b in range(2)], geoPT)
        modality_proj(semT[:], SEM, ws_sb, brow_sb[1][:],
                      [gb_sb[b][:, 2:3] for b in range(2)], [gb_sb[b][:, 3:4] for b in range(2)], semPT)
        modality_proj(rsaT[:], RSA, wr_sb, brow_sb[2][:],
                      [gb_sb[b][:, 4:5] for b in range(2)], [gb_sb[b][:, 5:6] for b in range(2)], rsaPT)

        # =====================================================================
        # Stage 3: WvWo fold, Q/K/V projections, KV table [k1|k2|v1|v2], AllGather
        # =====================================================================
        def rows_mm(srcPT, w_chunks, bias_row, t):
            o_ps = ps_acc.tile([128, D], FP32, tag="qkv_ps", name="qkv_ps")
            for kc in range(2):
                nc.tensor.matmul(o_ps[:], srcPT[kc][:, t * 128:(t + 1) * 128], w_chunks[kc][:],
                                 start=(kc == 0), stop=False)
            nc.tensor.matmul(o_ps[:], ones_1[:], bias_row, start=False, stop=True)
            return o_ps

        q1t, q2t = [], []
        for t in range(NT):
            ps = rows_mm(geoPT, wq_sb, brow_sb[3][:], t)
            q1 = pc.tile([128, D], BF16, tag=f"q1_{t}", name=f"q1_{t}")
            nc.scalar.activation(out=q1[:], in_=ps[:], func=AF.Copy, scale=QSCALE)
            q1t.append(q1)
            ps = rows_mm(semPT, wq_sb, brow_sb[3][:], t)
            q2 = pc.tile([128, D], BF16, tag=f"q2_{t}", name=f"q2_{t}")
            nc.scalar.activation(out=q2[:], in_=ps[:], func=AF.Copy, scale=QSCALE)
            q2t.append(q2)

            kv = pw2.tile([128, 4 * D], BF16, tag="kv", name="kv")
            ps = rows_mm(semPT, wk_sb, brow_sb[4][:], t)
            nc.any.tensor_copy(kv[:, 0:D], ps[:])
            ps = rows_mm(geoPT, wk_sb, brow_sb[4][:], t)
            nc.any.tensor_copy(kv[:, D:2 * D], ps[:])
            ps = rows_mm(semPT, wv_sb, brow_sb[6][:], t)
            nc.any.tensor_copy(kv[:, 2 * D:3 * D], ps[:])
            ps = rows_mm(geoPT, wv_sb, brow_sb[6][:], t)
            nc.any.tensor_copy(kv[:, 3 * D:4 * D], ps[:])
            nc.sync.dma_start(out=cc_in[t * 128:(t + 1) * 128, :], in_=kv[:])

        nc.gpsimd.collective_compute(
            "AllGather", ALU.bypass,
            replica_groups=[list(range(N_CORES))],
            ins=[cc_in[:]], outs=[cc_out[:]],
        )

        # =====================================================================
        # Stage 4: rsa transform (in [d, q]) -> combinedT rows 512..767
        # =====================================================================
        combT = [pc.tile([128, R], FP32, tag=f"combT{i}", name=f"combT{i}") for i in range(6)]
        r_xs, r_sqs, r_outs = [], [], []
        for mb in range(2):
            p_ps = ps_acc.tile([128, R], FP32, tag=f"proj_ps{mb}", name=f"rsa2_ps{mb}")
            for kc in range(2):
                nc.tensor.matmul(p_ps[:], wt_sb[kc][:, mb * 128:(mb + 1) * 128], rsaPT[kc][:],
                                 start=(kc == 0), stop=False)
            nc.tensor.matmul(p_ps[:], brow_sb[5][:, mb * 128:(mb + 1) * 128], ones_row[:],
                             start=False, stop=True)
            x = pw.tile([128, R], FP32, tag=f"mp_x{mb}", name=f"rsa2_x{mb}")
            nc.any.tensor_copy(x[:], p_ps[:])
            sq = pw.tile([128, R], FP32, tag=f"mp_sq{mb}", name=f"rsa2_sq{mb}")
            o = pw.tile([128, R], FP32, tag=f"rsa2_ln{mb}", name=f"rsa2_ln{mb}")
            r_xs.append(x); r_sqs.append(sq); r_outs.append(o)
        ln_dq(r_xs, r_sqs, [gb_sb[b][:, 6:7] for b in range(2)],
              [gb_sb[b][:, 7:8] for b in range(2)], r_outs, 2)
        for mb in range(2):
            lx = pw.tile([128, R], FP32, tag="rsa_leak", name="rsa_leak")
            nc.vector.tensor_scalar_mul(lx[:], r_outs[mb][:], 0.01)
            nc.vector.tensor_max(out=combT[4 + mb][:], in0=r_outs[mb][:], in1=lx[:])

        # =====================================================================
        # Stage 5+6: per-tile attention, residual LNs, FFN
        # =====================================================================
        hT = [pc.tile([128, R], FP32, tag=f"hT{i}", name=f"hT{i}") for i in range(4)]

        def transpose_to(dst_ap, src_ap):
            tp = ps_rep.tile([128, 128], FP32, tag="lnd_reps", name="transp")
            nc.tensor.transpose(out=tp[:], in_=src_ap, identity=ident[:])
            nc.any.tensor_copy(dst_ap, tp[:])

        for t in range(NT):
            g_sb = pg.tile([128, K, 4 * D], BF16, tag="g_sb", name="g_sb", bufs=1)
            for n in range(K):
                nc.gpsimd.indirect_dma_start(
                    out=g_sb[:, n, :], out_offset=None, in_=cc_out[:],
                    in_offset=bass.IndirectOffsetOnAxis(ap=idxcat[t][:, n:n + 1], axis=0),
                )

            def attn(qtile, off_k, off_v):
                prod = pw2.tile([128, K * D], BF16, tag="prod", name="prod")
                qb = qtile[:].rearrange("p (r e) -> p r e", r=1)
                nc.vector.tensor_tensor(out=prod[:], in0=g_sb[:, :, off_k:off_k + D],
                                        in1=qb.to_broadcast([128, K, D]), op=ALU.mult)
                s = pw.tile([128, K * H], FP32, tag="s_nh", name="s_nh")  # [n][h]
                nc.vector.tensor_reduce(out=s[:], in_=prod[:].rearrange("p (g d) -> p g d", d=HD),
                                        axis=AX.X, op=ALU.add)
                mx = pw.tile([128, H], FP32, tag="s_mx", name="s_mx")
                s_hn = s[:].rearrange("p (n h) -> p h n", h=H)
                nc.vector.tensor_reduce(out=mx[:], in_=s_hn, axis=AX.X, op=ALU.max)
                z = pw.tile([128, H * K], FP32, tag="s_z", name="s_z")   # [h][n]
                mxb = mx[:].rearrange("p (h r) -> p h r", r=1)
                nc.vector.tensor_tensor(out=z[:], in0=s_hn, in1=mxb.to_broadcast([128, H, K]), op=ALU.subtract)
                w = pw.tile([128, H * K], FP32, tag="s_w", name="s_w")
                nc.scalar.activation(out=w[:], in_=z[:], func=AF.Exp)
                sm = pw.tile([128, H], FP32, tag="s_sm", name="s_sm")
                nc.vector.tensor_reduce(out=sm[:], in_=w[:].rearrange("p (h n) -> p h n", h=H),
                                        axis=AX.X, op=ALU.add)
                rec = pw.tile([128, H], FP32, tag="s_rec", name="s_rec")
                nc.vector.reciprocal(out=rec[:], in_=sm[:])
                recb = rec[:].rearrange("p (h r) -> p h r", r=1)
                nc.vector.tensor_tensor(out=w[:], in0=w[:], in1=recb.to_broadcast([128, H, K]), op=ALU.mult)
                prod2 = pw2.tile([128, D * K], BF16, tag="prod", name="prod2")
                vview = g_sb[:, :, off_v:off_v + D].rearrange("p n (h d) -> p n h d", d=HD)
                wview = w[:].rearrange("p (h n o) -> p n h o", h=H, o=1)
                p2view = prod2[:].rearrange("p (h d n) -> p n h d", h=H, d=HD)
                nc.vector.tensor_tensor(out=p2view, in0=vview, in1=wview.to_broadcast([128, K, H, HD]), op=ALU.mult)
                o = pw.tile([128, D], FP32, tag="attn_o", name="attn_o")
                nc.vector.tensor_reduce(out=o[:], in_=prod2[:].rearrange("p (g n) -> p g n", n=K),
                                        axis=AX.X, op=ALU.add)
                oT = pw.tile([128, D], FP32, tag="oT", name="oT")
                for mb in range(2):
                    transpose_to(oT[:, mb * 128:(mb + 1) * 128], o[:, mb * 128:(mb + 1) * 128])
                op_ps = ps_acc.tile([128, D], FP32, tag="qkv_ps", name="op_ps")
                for kc in range(2):
                    nc.tensor.matmul(op_ps[:], oT[:, kc * 128:(kc + 1) * 128], wo_sb[kc][:],
                                     start=(kc == 0), stop=(kc == 1))
                ap = pw.tile([128, D], FP32, tag="attn_p", name="attn_p")
                nc.any.tensor_copy(ap[:], op_ps[:])
                return ap

            def residual_ln(attn_o, p_qd, a_col, be_col, g_rep, b_rep, out_ap):
                t1 = pw.tile([128, D], FP32, tag="res_t1", name="res_t1")
                nc.vector.tensor_tensor(out=t1[:], in0=attn_o[:], in1=bo_sb[:], op=ALU.add)
                nc.vector.tensor_scalar(t1[:], t1[:], be_col, None, op0=ALU.mult)
                t2 = pw.tile([128, D], FP32, tag="res_t2", name="res_t2")
                nc.vector.tensor_scalar(t2[:], p_qd, a_col, None, op0=ALU.mult)
                nc.vector.tensor_tensor(out=t1[:], in0=t1[:], in1=t2[:], op=ALU.add)
                ln_qd(t1[:], g_rep, b_rep, out_ap, D)

            attn1 = attn(q1t[t], 0, 2 * D)
            geo_qd = pw.tile([128, D], FP32, tag="geo_qd", name="geo_qd")
            for mb in range(2):
                transpose_to(geo_qd[:, mb * 128:(mb + 1) * 128], geoPT[mb][:, t * 128:(t + 1) * 128])
            geo_out = pw.tile([128, D], FP32, tag="geo_out", name="geo_out")
            residual_ln(attn1, geo_qd[:], scal_sb[:, 0:1], scal_sb[:, 1:2],
                        g1b1_sb[:, 0:D], g1b1_sb[:, D:2 * D], geo_out[:])
            for mb in range(2):
                transpose_to(combT[mb][:, t * 128:(t + 1) * 128], geo_out[:, mb * 128:(mb + 1) * 128])

            attn2 = attn(q2t[t], D, 3 * D)
            sem_qd = pw.tile([128, D], FP32, tag="sem_qd", name="sem_qd")
            for mb in range(2):
                transpose_to(sem_qd[:, mb * 128:(mb + 1) * 128], semPT[mb][:, t * 128:(t + 1) * 128])
            sem_out = pw.tile([128, D], FP32, tag="sem_out", name="sem_out")
            residual_ln(attn2, sem_qd[:], scal_sb[:, 2:3], scal_sb[:, 3:4],
                        g1b1_sb[:, 2 * D:3 * D], g1b1_sb[:, 3 * D:4 * D], sem_out[:])
            for mb in range(2):
                transpose_to(combT[2 + mb][:, t * 128:(t + 1) * 128], sem_out[:, mb * 128:(mb + 1) * 128])

            # ---- FFN layer 1 ----
            f1_ps = ps_small.tile([128, 2 * D], FP32, tag="c_ps", name="f1_ps")
            for kc in range(6):
                nc.tensor.matmul(f1_ps[:], combT[kc][:, t * 128:(t + 1) * 128], wf1_sb[kc][:],
                                 start=(kc == 0), stop=False)
            nc.tensor.matmul(f1_ps[:], ones_1[:], bf1_sb[:], start=False, stop=True)
            f1x = pw.tile([128, 2 * D], FP32, tag="f1x", name="f1x")
            nc.any.tensor_copy(f1x[:], f1_ps[:])
            f1ln = pw.tile([128, 2 * D], FP32, tag="f1x", name="f1ln")
            ln_qd(f1x[:], gbf1_sb[:, 0:2 * D], gbf1_sb[:, 2 * D:4 * D], f1ln[:], 2 * D)
            h_t = pw.tile([128, 2 * D], FP32, tag="h_t", name="h_t")
            leaky(f1ln[:], h_t[:], 2 * D)
            for mb in range(4):
                transpose_to(hT[mb][:, t * 128:(t + 1) * 128], h_t[:, mb * 128:(mb + 1) * 128])

            # ---- FFN layer 2 ----
            f2_ps = ps_acc.tile([128, D], FP32, tag="qkv_ps", name="f2_ps")
            for kc in range(4):
                nc.tensor.matmul(f2_ps[:], hT[kc][:, t * 128:(t + 1) * 128], wf2_sb[kc][:],
                                 start=(kc == 0), stop=False)
            nc.tensor.matmul(f2_ps[:], ones_1[:], brow_sb[7][:], start=False, stop=True)
            f2x = pw.tile([128, D], FP32, tag="f2x", name="f2x")
            nc.any.tensor_copy(f2x[:], f2_ps[:])
            f2ln = pw.tile([128, D], FP32, tag="f2x", name="f2ln")
            ln_qd(f2x[:], gbf2_sb[:, 0:D], gbf2_sb[:, D:2 * D], f2ln[:], D)
            fout = pw.tile([128, D], FP32, tag="res_t1", name="fout")
            leaky(f2ln[:], fout[:], D)
            nc.sync.dma_start(out=out_fused[t * 128:(t + 1) * 128, :], in_=fout[:])

    nc.compile()
    return nc


def make_in_maps(geo_feat, sem_feat, rsa_feat, pos, params):
    p = params
    f32 = np.float32
    def T(x):
        return np.ascontiguousarray(np.asarray(x, dtype=f32).T)

    posT = T(pos)
    brow = np.stack([np.asarray(p[k], f32) for k in
                     ("bg", "bs", "br", "bq", "bk", "bt", "bv", "bf2")])
    gb_col = np.stack([np.asarray(p[k], f32) for k in
                       ("gg", "bgn", "gs", "bsn", "gr", "brn", "gt", "btn")], axis=1)
    g1b1 = np.concatenate([np.tile(np.asarray(p[k], f32)[None, :], (128, 1))
                           for k in ("g1", "b1", "g2", "b2")], axis=1)
    gbf1 = np.concatenate([np.tile(np.asarray(p[k], f32)[None, :], (128, 1))
                           for k in ("gf1", "bf1n")], axis=1)
    gbf2 = np.concatenate([np.tile(np.asarray(p[k], f32)[None, :], (128, 1))
                           for k in ("gf2", "bf2n")], axis=1)
    bo_rep = np.tile(np.asarray(p["bo"], f32)[None, :], (128, 1))
    scal = np.tile(np.array([p["a_g"], p["be_g"], p["a_s"], p["be_s"]], f32)[None, :], (128, 1))

    shared = {
        "posT": posT,
        "Wg": np.asarray(p["Wg"], f32), "Ws": np.asarray(p["Ws"], f32), "Wr": np.asarray(p["Wr"], f32),
        "Wq": np.asarray(p["Wq"], f32), "Wk": np.asarray(p["Wk"], f32),
        "Wv": np.asarray(p["Wv"], f32), "Wo": np.asarray(p["Wo"], f32), "Wt": np.asarray(p["Wt"], f32),
        "Wf1": np.asarray(p["Wf1"], f32), "Wf2": np.asarray(p["Wf2"], f32),
        "brow": np.ascontiguousarray(brow), "bf1_row": np.asarray(p["bf1"], f32)[None, :],
        "gb_col": np.ascontiguousarray(gb_col),
        "g1b1": g1b1, "gbf1": gbf1, "gbf2": gbf2, "bo_rep": bo_rep, "scal": scal,
    }
    in_maps = []
    for c in range(N_CORES):
        r0 = c * R
        m = dict(shared)
        m["geoT"] = T(np.asarray(geo_feat)[r0:r0 + R])
        m["semT"] = T(np.asarray(sem_feat)[r0:r0 + R])
        m["rsaT"] = T(np.asarray(rsa_feat)[r0:r0 + R])
        m["posrT"] = np.ascontiguousarray(posT[:, r0:r0 + R])
        in_maps.append(m)
    return in_maps


def kernel(geo_feat, sem_feat, rsa_feat, pos, params):
    if "nc" not in _CACHE:
        _CACHE["nc"] = build_program()
    nc = _CACHE["nc"]
    in_maps = make_in_maps(geo_feat, sem_feat, rsa_feat, pos, params)

    if os.environ.get("KERNEL_SIM"):
        from concourse.bass_interp import MultiCoreSim
        sim = MultiCoreSim(nc, num_cores=N_CORES, trace=False)
        for c in range(N_CORES):
            core = sim.cores[c]
            for k, v in in_maps[c].items():
                core.tensor(k)[:] = v
        sim.simulate(check_with_hw=False)
        outs = [np.array(sim.cores[c].tensor("fused")) for c in range(N_CORES)]
    else:
        res = bass_utils.run_bass_kernel_spmd(nc, in_maps, list(range(N_CORES)))
        outs = [np.asarray(res.results[c]["fused"]) for c in range(N_CORES)]
    return np.concatenate(outs, axis=0)


# revision 21
# speedup vs baseline: 94.8599x; 45.5954x over previous
"""Trainium2 Bass kernel for nn_LocalCrossModalAttention (sparse k-NN cross attention).

Sharding: rows (queries) L=3072 split across 8 cores (384 each). K/V projections
(with Wo folded into V) are computed row-parallel, AllGathered as a bf16
[3072, 1024] table, then each core gathers its queries' 10 nearest neighbors'
K/V rows via indirect DMA and does the sparse attention + FFN for its rows.
"""
import os
import numpy as np

import concourse.bass as bass
import concourse.mybir as mybir
from concourse import bass_utils, bacc
from concourse.tile import TileContext

FP32 = mybir.dt.float32
BF16 = mybir.dt.bfloat16
U32 = mybir.dt.uint32
AF = mybir.ActivationFunctionType
ALU = mybir.AluOpType
AX = mybir.AxisListType

N_CORES = 8
L, GEO, SEM, RSA, D, H, HD, K = 3072, 1536, 512, 64, 256, 8, 32, 10
R = L // N_CORES           # rows per core = 384
NT = R // 128              # query tiles per core = 3
EPS = 1e-5
QSCALE = 1.0 / np.sqrt(HD)

_CACHE = {}


def build_program():
    nc = bacc.Bacc("TRN2", target_bir_lowering=False, debug=False, num_devices=N_CORES)

    # ---- I/O -------------------------------------------------------------
    def inp(name, shape, dtype=FP32):
        return nc.declare_dram_parameter(name, list(shape), dtype, isOutput=False)

    geoT = inp("geoT", [GEO, R])
    semT = inp("semT", [SEM, R])
    rsaT = inp("rsaT", [RSA, R])
    posT = inp("posT", [3, L])
    posrT = inp("posrT", [3, R])
    wg = inp("Wg", [GEO, D]); ws = inp("Ws", [SEM, D]); wr = inp("Wr", [RSA, D])
    wq = inp("Wq", [D, D]); wk = inp("Wk", [D, D])
    wv = inp("Wv", [D, D]); wo = inp("Wo", [D, D]); wt = inp("Wt", [D, D])
    wf1 = inp("Wf1", [3 * D, 2 * D]); wf2 = inp("Wf2", [2 * D, D])
    brow = inp("brow", [8, D])            # bg, bs, br, bq, bk, bt, bv, bf2 rows
    bf1_row = inp("bf1_row", [1, 2 * D])
    gb_col = inp("gb_col", [D, 8])        # cols: gg,bgn,gs,bsn,gr,brn,gt,btn
    g1b1 = inp("g1b1", [128, 4 * D])      # g1,b1,g2,b2 replicated
    gbf1 = inp("gbf1", [128, 4 * D])      # gf1,bf1n replicated (512 each)
    gbf2 = inp("gbf2", [128, 2 * D])      # gf2,bf2n replicated
    bo_rep = inp("bo_rep", [128, D])      # bo replicated
    scal = inp("scal", [128, 4])          # a_g, be_g, a_s, be_s columns

    out_fused = nc.declare_dram_parameter("fused", [R, D], FP32, isOutput=True)

    cc_in = nc.dram_tensor("cc_in", [R, 4 * D], BF16)
    cc_out = nc.dram_tensor("cc_out", [L, 4 * D], BF16, addr_space="Shared")

    from contextlib import ExitStack
    with TileContext(nc) as tc, ExitStack() as stack:
        pc = stack.enter_context(tc.tile_pool(name="const", bufs=1))
        pw = stack.enter_context(tc.tile_pool(name="work", bufs=1))
        pw2 = stack.enter_context(tc.tile_pool(name="work2", bufs=2))
        pg = stack.enter_context(tc.tile_pool(name="gath", bufs=2))
        ps_acc = stack.enter_context(tc.tile_pool(name="ps_acc", bufs=1, space="PSUM"))
        ps_small = stack.enter_context(tc.tile_pool(name="ps_small", bufs=1, space="PSUM"))
        ps_rep = stack.enter_context(tc.tile_pool(name="ps_rep", bufs=1, space="PSUM"))

        # ---- persistent weights/params ------------------------------------
        def load_chunks(src, rows, cols, tag):
            tiles = []
            for kc in range((rows + 127) // 128):
                p0 = kc * 128
                pn = min(128, rows - p0)
                t = pc.tile([pn, cols], FP32, tag=f"{tag}{kc}", name=f"{tag}{kc}")
                nc.sync.dma_start(out=t[:], in_=src[p0:p0 + pn, :])
                tiles.append(t)
            return tiles

        wg_sb = load_chunks(wg[:], GEO, D, "wg")
        ws_sb = load_chunks(ws[:], SEM, D, "ws")
        wr_sb = load_chunks(wr[:], RSA, D, "wr")
        wq_sb = load_chunks(wq[:], D, D, "wq")
        wk_sb = load_chunks(wk[:], D, D, "wk")
        wv_sb = load_chunks(wv[:], D, D, "wv")
        wo_sb = load_chunks(wo[:], D, D, "wo")
        wt_sb = load_chunks(wt[:], D, D, "wt")
        def load_chunks_tags(src, rows, cols, tags, names):
            tiles = []
            for kc in range((rows + 127) // 128):
                p0 = kc * 128
                pn = min(128, rows - p0)
                t = pc.tile([pn, cols], FP32, tag=tags[kc], name=names[kc])
                nc.sync.dma_start(out=t[:], in_=src[p0:p0 + pn, :])
                tiles.append(t)
            return tiles

        wf1_sb = load_chunks_tags(wf1[:], 3 * D, 2 * D,
                                  [f"wg{i}" for i in range(6)], [f"wf1_{i}" for i in range(6)])
        wf2_sb = load_chunks_tags(wf2[:], 2 * D, D,
                                  [f"wg{i}" for i in range(6, 10)], [f"wf2_{i}" for i in range(4)])

        brow_sb = []
        for bi in range(8):
            t = pc.tile([1, D], FP32, tag=f"brow{bi}", name=f"brow{bi}")
            nc.sync.dma_start(out=t[:], in_=brow[bi:bi + 1, :])
            brow_sb.append(t)
        bf1_sb = pc.tile([1, 2 * D], FP32, tag="bf1")
        nc.sync.dma_start(out=bf1_sb[:], in_=bf1_row[:])
        gb_sb = load_chunks(gb_col[:], D, 8, "gbcol")
        g1b1_sb = pc.tile([128, 4 * D], FP32, tag="g1b1")
        nc.sync.dma_start(out=g1b1_sb[:], in_=g1b1[:])
        gbf1_sb = pc.tile([128, 4 * D], FP32, tag="gbf1")
        nc.sync.dma_start(out=gbf1_sb[:], in_=gbf1[:])
        gbf2_sb = pc.tile([128, 2 * D], FP32, tag="gbf2")
        nc.sync.dma_start(out=gbf2_sb[:], in_=gbf2[:])
        bo_sb = pc.tile([128, D], FP32, tag="bo_rep")
        nc.sync.dma_start(out=bo_sb[:], in_=bo_rep[:])
        scal_sb = pc.tile([128, 4], FP32, tag="scal")
        nc.sync.dma_start(out=scal_sb[:], in_=scal[:])
        posrT_sb = pc.tile([3, R], FP32, tag="posrT")
        nc.sync.dma_start(out=posrT_sb[:], in_=posrT[:])

        ones_row = pc.tile([1, R], FP32, tag="ones_row")
        nc.vector.memset(ones_row[:], 1.0)
        ones_1 = pc.tile([1, 128], FP32, tag="ones_1p")
        nc.vector.memset(ones_1[:], 1.0)
        ones_d = pc.tile([128, 1], FP32, tag="ones_d")
        nc.vector.memset(ones_d[:], 1.0)
        ones_3 = pc.tile([3, 1], FP32, tag="ones_3")
        nc.vector.memset(ones_3[:], 1.0)
        from concourse.masks import make_identity
        ident = pc.tile([128, 128], FP32, tag="ident")
        make_identity(nc, ident[:])
        eps1 = pc.tile([1, 1], FP32, tag="eps1")
        nc.vector.memset(eps1[:], EPS)
        eps128 = pc.tile([128, 1], FP32, tag="eps128")
        nc.vector.memset(eps128[:], EPS)

        # ---- LN helpers ---------------------------------------------------
        def ln_dq(x_sb, sq_sb, g_col, b_col, out_blocks, nblk):
            """LayerNorm in [d, q]: x/sq/out are lists of [128, R] tiles."""
            dsz = nblk * 128
            mean_ps = ps_small.tile([1, R], FP32, tag="lnd_mean", name="lnd_mean")
            var_ps = ps_small.tile([1, R], FP32, tag="lnd_var", name="lnd_var")
            for b in range(nblk):
                nc.scalar.activation(out=sq_sb[b][:], in_=x_sb[b][:], func=AF.Square)
            for b in range(nblk):
                nc.tensor.matmul(mean_ps[:], ones_d[:], x_sb[b][:], start=(b == 0), stop=(b == nblk - 1))
            for b in range(nblk):
                nc.tensor.matmul(var_ps[:], ones_d[:], sq_sb[b][:], start=(b == 0), stop=(b == nblk - 1))
            negmean = pw.tile([1, R], FP32, tag="lnd_negmean", name="lnd_negmean")
            e2 = pw.tile([1, R], FP32, tag="lnd_e2", name="lnd_e2")
            msq = pw.tile([1, R], FP32, tag="lnd_msq", name="lnd_msq")
            var = pw.tile([1, R], FP32, tag="lnd_varsb", name="lnd_varsb")
            rstd = pw.tile([1, R], FP32, tag="lnd_rstd", name="lnd_rstd")
            nc.scalar.activation(out=negmean[:], in_=mean_ps[:], func=AF.Copy, scale=-1.0 / dsz)
            nc.scalar.activation(out=e2[:], in_=var_ps[:], func=AF.Copy, scale=1.0 / dsz)
            nc.scalar.activation(out=msq[:], in_=negmean[:], func=AF.Square)
            nc.vector.tensor_tensor(out=var[:], in0=e2[:], in1=msq[:], op=ALU.subtract)
            nc.scalar.activation(out=var[:], in_=var[:], func=AF.Sqrt, bias=eps1[:])
            nc.vector.reciprocal(out=rstd[:], in_=var[:])
            repm = ps_rep.tile([128, R], FP32, tag="lnd_repm", name="lnd_repm")
            reps = ps_rep.tile([128, R], FP32, tag="lnd_reps", name="lnd_reps")
            nc.tensor.matmul(repm[:], ones_1[:], negmean[:], start=True, stop=True)
            nc.tensor.matmul(reps[:], ones_1[:], rstd[:], start=True, stop=True)
            for b in range(nblk):
                u = pw.tile([128, R], FP32, tag="lnd_u", name="lnd_u")
                nc.vector.tensor_tensor(out=u[:], in0=x_sb[b][:], in1=repm[:], op=ALU.add)
                nc.vector.tensor_tensor(out=u[:], in0=u[:], in1=reps[:], op=ALU.mult)
                nc.vector.tensor_scalar(out_blocks[b][:], u[:], g_col[b], b_col[b], op0=ALU.mult, op1=ALU.add)

        def ln_qd(r_ap, g_rep, b_rep, out_ap, dsz):
            mean = pw.tile([128, 1], FP32, tag="lnq_mean", name="lnq_mean")
            nc.vector.tensor_reduce(out=mean[:], in_=r_ap, axis=AX.X, op=ALU.add)
            negmean = pw.tile([128, 1], FP32, tag="lnq_negmean", name="lnq_negmean")
            nc.scalar.activation(out=negmean[:], in_=mean[:], func=AF.Copy, scale=-1.0 / dsz)
            xm = pw.tile([128, 2 * D], FP32, tag="lnq_xm", name="lnq_xm")[:, 0:dsz]
            nc.vector.tensor_scalar(xm, r_ap, negmean[:], None, op0=ALU.add)
            sq = pw.tile([128, 2 * D], FP32, tag="lnq_sq", name="lnq_sq")[:, 0:dsz]
            nc.scalar.activation(out=sq, in_=xm, func=AF.Square)
            ssum = pw.tile([128, 1], FP32, tag="lnq_ssum", name="lnq_ssum")
            nc.vector.tensor_reduce(out=ssum[:], in_=sq, axis=AX.X, op=ALU.add)
            std = pw.tile([128, 1], FP32, tag="lnq_std", name="lnq_std")
            nc.scalar.activation(out=std[:], in_=ssum[:], func=AF.Sqrt, scale=1.0 / dsz, bias=eps128[:])
            rec = pw.tile([128, 1], FP32, tag="lnq_rec", name="lnq_rec")
            nc.vector.reciprocal(out=rec[:], in_=std[:])
            nc.vector.tensor_scalar(xm, xm, rec[:], None, op0=ALU.mult)
            nc.vector.tensor_tensor(out=xm, in0=xm, in1=g_rep, op=ALU.mult)
            nc.vector.tensor_tensor(out=out_ap, in0=xm, in1=b_rep, op=ALU.add)

        def leaky(x_ap, out_ap, dsz):
            lx = pw.tile([128, 2 * D], FP32, tag="lnq_sq", name="leaky_lx")[:, 0:dsz]
            nc.vector.tensor_scalar_mul(lx, x_ap, 0.01)
            nc.vector.tensor_max(out=out_ap, in0=x_ap, in1=lx)

        # =====================================================================
        # Stage 1: k-NN top-10  (C[i,j] = 2 p_i . p_j - |p_j|^2)
        # =====================================================================
        rhs3 = pc.tile([3, L], FP32, tag="rhs3")
        njneg = pc.tile([1, L], FP32, tag="njneg")
        for ch in range(L // 512):
            sl = slice(ch * 512, (ch + 1) * 512)
            pch = pw2.tile([3, 512], FP32, tag="pos_ch", name="pos_ch")
            nc.sync.dma_start(out=pch[:], in_=posT[:, sl])
            nc.vector.tensor_copy(rhs3[:, sl], pch[:])
            sqch = pw2.tile([3, 512], FP32, tag="sq_ch", name="sq_ch")
            nc.scalar.activation(out=sqch[:], in_=pch[:], func=AF.Square)
            nj_ps = ps_small.tile([1, 512], FP32, tag="lnd_mean", name="nj_ps")
            nc.tensor.matmul(nj_ps[:], ones_3[:], sqch[:], start=True, stop=True)
            nc.scalar.activation(out=njneg[:, sl], in_=nj_ps[:], func=AF.Copy, scale=-1.0)
        pos3 = pc.tile([3, R], FP32, tag="pos3")
        nc.vector.tensor_scalar_mul(pos3[:], posrT_sb[:], 2.0)

        idxcat = []
        for t in range(NT):
            d2n = pw2.tile([128, L], FP32, tag="d2n", name="d2n", bufs=1)
            for ch in range(L // 512):
                c_ps = ps_small.tile([128, 512], FP32, tag="c_ps", name="c_ps")
                nc.tensor.matmul(c_ps[:], pos3[:, t * 128:(t + 1) * 128],
                                 rhs3[:, ch * 512:(ch + 1) * 512], start=True, stop=False)
                nc.tensor.matmul(c_ps[:], ones_row[:, t * 128:(t + 1) * 128],
                                 njneg[:, ch * 512:(ch + 1) * 512], start=False, stop=True)
                nc.any.tensor_copy(d2n[:, ch * 512:(ch + 1) * 512], c_ps[:])
            m8 = pw.tile([128, 16], FP32, tag="m8", name="m8")
            idx = pc.tile([128, 16], U32, tag=f"idx{t}", name=f"idx{t}")
            nc.vector.max(out=m8[:, 0:8], in_=d2n[:])
            nc.vector.max_index(out=idx[:, 0:8], in_max=m8[:, 0:8], in_values=d2n[:])
            nc.vector.match_replace(out=d2n[:], in_to_replace=m8[:, 0:8], in_values=d2n[:], imm_value=-3e38)
            nc.vector.max(out=m8[:, 8:16], in_=d2n[:])
            nc.vector.max_index(out=idx[:, 8:16], in_max=m8[:, 8:16], in_values=d2n[:])
            idxcat.append(idx)

        # =====================================================================
        # Stage 2: modality projections (projT [d, q]) + LN in [d, q]
        # =====================================================================
        def modality_proj(featT_dram, nfeat, w_sb, bias_row, g_cols, b_cols, out_tiles):
            nkc = (nfeat + 127) // 128
            pss = [ps_acc.tile([128, R], FP32, tag=f"proj_ps{mb}", name=f"proj_ps{mb}")
                   for mb in range(2)]
            for kc in range(nkc):
                p0 = kc * 128
                pn = min(128, nfeat - p0)
                fch = pw2.tile([128, R], FP32, tag="featT", name="featT")
                nc.sync.dma_start(out=fch[:pn, :], in_=featT_dram[p0:p0 + pn, :])
                for mb in range(2):
                    nc.tensor.matmul(pss[mb][:], w_sb[kc][:, mb * 128:(mb + 1) * 128], fch[:pn, :],
                                     start=(kc == 0), stop=False)
            xs, sqs = [], []
            for mb in range(2):
                nc.tensor.matmul(pss[mb][:], bias_row[:, mb * 128:(mb + 1) * 128], ones_row[:],
                                 start=False, stop=True)
                x = pw.tile([128, R], FP32, tag=f"mp_x{mb}", name=f"mp_x{mb}")
                nc.any.tensor_copy(x[:], pss[mb][:])
                sq = pw.tile([128, R], FP32, tag=f"mp_sq{mb}", name=f"mp_sq{mb}")
                xs.append(x); sqs.append(sq)
            ln_dq(xs, sqs, g_cols, b_cols, out_tiles, 2)

        geoPT = [pc.tile([128, R], FP32, tag=f"geoPT{b}", name=f"geoPT{b}") for b in range(2)]
        semPT = [pc.tile([128, R], FP32, tag=f"semPT{b}", name=f"semPT{b}") for b in range(2)]
        rsaPT = [pw.tile([128, R], FP32, tag=f"rsaPT{b}", name=f"rsaPT{b}") for b in range(2)]
        modality_proj(geoT[:], GEO, wg_sb, brow_sb[0][:],
                      [gb_sb[b][:, 0:1] for b in range(2)], [gb_sb[b][:, 1:2] for b in range(2)], geoPT)
        modality_proj(semT[:], SEM, ws_sb, brow_sb[1][:],
                      [gb_sb[b][:, 2:3] for b in range(2)], [gb_sb[b][:, 3:4] for b in range(2)], semPT)
        modality_proj(rsaT[:], RSA, wr_sb, brow_sb[2][:],
                      [gb_sb[b][:, 4:5] for b in range(2)], [gb_sb[b][:, 5:6] for b in range(2)], rsaPT)

        # =====================================================================
        # Stage 3: WvWo fold, Q/K/V projections, KV table [k1|k2|v1|v2], AllGather
        # =====================================================================
        def rows_mm(srcPT, w_chunks, bias_row, t):
            o_ps = ps_acc.tile([128, D], FP32, tag="qkv_ps", name="qkv_ps")
            for kc in range(2):
                nc.tensor.matmul(o_ps[:], srcPT[kc][:, t * 128:(t + 1) * 128], w_chunks[kc][:],
                                 start=(kc == 0), stop=False)
            nc.tensor.matmul(o_ps[:], ones_1[:], bias_row, start=False, stop=True)
            return o_ps

        q1t, q2t = [], []
        for t in range(NT):
            ps = rows_mm(geoPT, wq_sb, brow_sb[3][:], t)
            q1 = pc.tile([128, D], BF16, tag=f"q1_{t}", name=f"q1_{t}")
            nc.scalar.activation(out=q1[:], in_=ps[:], func=AF.Copy, scale=QSCALE)
            q1t.append(q1)
            ps = rows_mm(semPT, wq_sb, brow_sb[3][:], t)
            q2 = pc.tile([128, D], BF16, tag=f"q2_{t}", name=f"q2_{t}")
            nc.scalar.activation(out=q2[:], in_=ps[:], func=AF.Copy, scale=QSCALE)
            q2t.append(q2)

            kv = pw2.tile([128, 4 * D], BF16, tag="kv", name="kv")
            ps = rows_mm(semPT, wk_sb, brow_sb[4][:], t)
            nc.any.tensor_copy(kv[:, 0:D], ps[:])
            ps = rows_mm(geoPT, wk_sb, brow_sb[4][:], t)
            nc.any.tensor_copy(kv[:, D:2 * D], ps[:])
            ps = rows_mm(semPT, wv_sb, brow_sb[6][:], t)
            nc.any.tensor_copy(kv[:, 2 * D:3 * D], ps[:])
            ps = rows_mm(geoPT, wv_sb, brow_sb[6][:], t)
            nc.any.tensor_copy(kv[:, 3 * D:4 * D], ps[:])
            nc.sync.dma_start(out=cc_in[t * 128:(t + 1) * 128, :], in_=kv[:])

        nc.gpsimd.collective_compute(
            "AllGather", ALU.bypass,
            replica_groups=[list(range(N_CORES))],
            ins=[cc_in[:]], outs=[cc_out[:]],
        )

        # =====================================================================
        # Stage 4: rsa transform (in [d, q]) -> combinedT rows 512..767
        # =====================================================================
        combT = [pc.tile([128, R], FP32, tag=f"combT{i}", name=f"combT{i}") for i in range(6)]
        r_xs, r_sqs, r_outs = [], [], []
        for mb in range(2):
            p_ps = ps_acc.tile([128, R], FP32, tag=f"proj_ps{mb}", name=f"rsa2_ps{mb}")
            for kc in range(2):
                nc.tensor.matmul(p_ps[:], wt_sb[kc][:, mb * 128:(mb + 1) * 128], rsaPT[kc][:],
                                 start=(kc == 0), stop=False)
            nc.tensor.matmul(p_ps[:], brow_sb[5][:, mb * 128:(mb + 1) * 128], ones_row[:],
                             start=False, stop=True)
            x = pw.tile([128, R], FP32, tag=f"mp_x{mb}", name=f"rsa2_x{mb}")
            nc.any.tensor_copy(x[:], p_ps[:])
            sq = pw.tile([128, R], FP32, tag=f"mp_sq{mb}", name=f"rsa2_sq{mb}")
            o = pw.tile([128, R], FP32, tag=f"rsa2_ln{mb}", name=f"rsa2_ln{mb}")
            r_xs.append(x); r_sqs.append(sq); r_outs.append(o)
        ln_dq(r_xs, r_sqs, [gb_sb[b][:, 6:7] for b in range(2)],
              [gb_sb[b][:, 7:8] for b in range(2)], r_outs, 2)
        for mb in range(2):
            lx = pw.tile([128, R], FP32, tag="rsa_leak", name="rsa_leak")
            nc.vector.tensor_scalar_mul(lx[:], r_outs[mb][:], 0.01)
            nc.vector.tensor_max(out=combT[4 + mb][:], in0=r_outs[mb][:], in1=lx[:])

        # =====================================================================
        # Stage 5+6: per-tile attention, residual LNs, FFN
        # =====================================================================
        hT = [pc.tile([128, R], FP32, tag=f"hT{i}", name=f"hT{i}") for i in range(4)]

        def transpose_to(dst_ap, src_ap):
            tp = ps_rep.tile([128, 128], FP32, tag="lnd_reps", name="transp")
            nc.tensor.transpose(out=tp[:], in_=src_ap, identity=ident[:])
            nc.any.tensor_copy(dst_ap, tp[:])

        for t in range(NT):
            g_sb = pg.tile([128, K, 4 * D], BF16, tag="g_sb", name="g_sb", bufs=1)
            for n in range(K):
                nc.gpsimd.indirect_dma_start(
                    out=g_sb[:, n, :], out_offset=None, in_=cc_out[:],
                    in_offset=bass.IndirectOffsetOnAxis(ap=idxcat[t][:, n:n + 1], axis=0),
                )

            def attn(qtile, off_k, off_v):
                prod = pw2.tile([128, K * D], BF16, tag="prod", name="prod")
                qb = qtile[:].rearrange("p (r e) -> p r e", r=1)
                nc.vector.tensor_tensor(out=prod[:], in0=g_sb[:, :, off_k:off_k + D],
                                        in1=qb.to_broadcast([128, K, D]), op=ALU.mult)
                s = pw.tile([128, K * H], FP32, tag="s_nh", name="s_nh")  # [n][h]
                nc.vector.tensor_reduce(out=s[:], in_=prod[:].rearrange("p (g d) -> p g d", d=HD),
                                        axis=AX.X, op=ALU.add)
                mx = pw.tile([128, H], FP32, tag="s_mx", name="s_mx")
                s_hn = s[:].rearrange("p (n h) -> p h n", h=H)
                nc.vector.tensor_reduce(out=mx[:], in_=s_hn, axis=AX.X, op=ALU.max)
                z = pw.tile([128, H * K], FP32, tag="s_z", name="s_z")   # [h][n]
                mxb = mx[:].rearrange("p (h r) -> p h r", r=1)
                nc.vector.tensor_tensor(out=z[:], in0=s_hn, in1=mxb.to_broadcast([128, H, K]), op=ALU.subtract)
                w = pw.tile([128, H * K], FP32, tag="s_w", name="s_w")
                nc.scalar.activation(out=w[:], in_=z[:], func=AF.Exp)
                sm = pw.tile([128, H], FP32, tag="s_sm", name="s_sm")
                nc.vector.tensor_reduce(out=sm[:], in_=w[:].rearrange("p (h n) -> p h n", h=H),
                                        axis=AX.X, op=ALU.add)
                rec = pw.tile([128, H], FP32, tag="s_rec", name="s_rec")
                nc.vector.reciprocal(out=rec[:], in_=sm[:])
                recb = rec[:].rearrange("p (h r) -> p h r", r=1)
                nc.vector.tensor_tensor(out=w[:], in0=w[:], in1=recb.to_broadcast([128, H, K]), op=ALU.mult)
                prod2 = pw2.tile([128, D * K], BF16, tag="prod", name="prod2")
                vview = g_sb[:, :, off_v:off_v + D].rearrange("p n (h d) -> p n h d", d=HD)
                wview = w[:].rearrange("p (h n o) -> p n h o", h=H, o=1)
                p2view = prod2[:].rearrange("p (h d n) -> p n h d", h=H, d=HD)
                nc.vector.tensor_tensor(out=p2view, in0=vview, in1=wview.to_broadcast([128, K, H, HD]), op=ALU.mult)
                o = pw.tile([128, D], FP32, tag="attn_o", name="attn_o")
                nc.vector.tensor_reduce(out=o[:], in_=prod2[:].rearrange("p (g n) -> p g n", n=K),
                                        axis=AX.X, op=ALU.add)
                oT = pw.tile([128, D], FP32, tag="oT", name="oT")
                for mb in range(2):
                    transpose_to(oT[:, mb * 128:(mb + 1) * 128], o[:, mb * 128:(mb + 1) * 128])
                op_ps = ps_acc.tile([128, D], FP32, tag="qkv_ps", name="op_ps")
                for kc in range(2):
                    nc.tensor.matmul(op_ps[:], oT[:, kc * 128:(kc + 1) * 128], wo_sb[kc][:],
                                     start=(kc == 0), stop=(kc == 1))
                ap = pw.tile([128, D], FP32, tag="attn_p", name="attn_p")
                nc.any.tensor_copy(ap[:], op_ps[:])
                return ap

            def residual_ln(attn_o, p_qd, a_col, be_col, g_rep, b_rep, out_ap):
                t1 = pw.tile([128, D], FP32, tag="res_t1", name="res_t1")
                nc.vector.tensor_tensor(out=t1[:], in0=attn_o[:], in1=bo_sb[:], op=ALU.add)
                nc.vector.tensor_scalar(t1[:], t1[:], be_col, None, op0=ALU.mult)
                t2 = pw.tile([128, D], FP32, tag="res_t2", name="res_t2")
                nc.vector.tensor_scalar(t2[:], p_qd, a_col, None, op0=ALU.mult)
                nc.vector.tensor_tensor(out=t1[:], in0=t1[:], in1=t2[:], op=ALU.add)
                ln_qd(t1[:], g_rep, b_rep, out_ap, D)

            attn1 = attn(q1t[t], 0, 2 * D)
            geo_qd = pw.tile([128, D], FP32, tag="geo_qd", name="geo_qd")
            for mb in range(2):
                transpose_to(geo_qd[:, mb * 128:(mb + 1) * 128], geoPT[mb][:, t * 128:(t + 1) * 128])
            geo_out = pw.tile([128, D], FP32, tag="geo_out", name="geo_out")
            residual_ln(attn1, geo_qd[:], scal_sb[:, 0:1], scal_sb[:, 1:2],
                        g1b1_sb[:, 0:D], g1b1_sb[:, D:2 * D], geo_out[:])
            for mb in range(2):
                transpose_to(combT[mb][:, t * 128:(t + 1) * 128], geo_out[:, mb * 128:(mb + 1) * 128])

            attn2 = attn(q2t[t], D, 3 * D)
            sem_qd = pw.tile([128, D], FP32, tag="sem_qd", name="sem_qd")
            for mb in range(2):
                transpose_to(sem_qd[:, mb * 128:(mb + 1) * 128], semPT[mb][:, t * 128:(t + 1) * 128])
            sem_out = pw.tile([128, D], FP32, tag="sem_out", name="sem_out")
            residual_ln(attn2, sem_qd[:], scal_sb[:, 2:3], scal_sb[:, 3:4],
                        g1b1_sb[:, 2 * D:3 * D], g1b1_sb[:, 3 * D:4 * D], sem_out[:])
            for mb in range(2):
                transpose_to(combT[2 + mb][:, t * 128:(t + 1) * 128], sem_out[:, mb * 128:(mb + 1) * 128])

            # ---- FFN layer 1 ----
            f1_ps = ps_small.tile([128, 2 * D], FP32, tag="c_ps", name="f1_ps")
            for kc in range(6):
                nc.tensor.matmul(f1_ps[:], combT[kc][:, t * 128:(t + 1) * 128], wf1_sb[kc][:],
                                 start=(kc == 0), stop=False)
            nc.tensor.matmul(f1_ps[:], ones_1[:], bf1_sb[:], start=False, stop=True)
            f1x = pw.tile([128, 2 * D], FP32, tag="f1x", name="f1x")
            nc.any.tensor_copy(f1x[:], f1_ps[:])
            f1ln = pw.tile([128, 2 * D], FP32, tag="f1x", name="f1ln")
            ln_qd(f1x[:], gbf1_sb[:, 0:2 * D], gbf1_sb[:, 2 * D:4 * D], f1ln[:], 2 * D)
            h_t = pw.tile([128, 2 * D], FP32, tag="h_t", name="h_t")
            leaky(f1ln[:], h_t[:], 2 * D)
            for mb in range(4):
                transpose_to(hT[mb][:, t * 128:(t + 1) * 128], h_t[:, mb * 128:(mb + 1) * 128])

            # ---- FFN layer 2 ----
            f2_ps = ps_acc.tile([128, D], FP32, tag="qkv_ps", name="f2_ps")
            for kc in range(4):
                nc.tensor.matmul(f2_ps[:], hT[kc][:, t * 128:(t + 1) * 128], wf2_sb[kc][:],
                                 start=(kc == 0), stop=False)
            nc.tensor.matmul(f2_ps[:], ones_1[:], brow_sb[7][:], start=False, stop=True)
            f2x = pw.tile([128, D], FP32, tag="f2x", name="f2x")
            nc.any.tensor_copy(f2x[:], f2_ps[:])
            f2ln = pw.tile([128, D], FP32, tag="f2x", name="f2ln")
            ln_qd(f2x[:], gbf2_sb[:, 0:D], gbf2_sb[:, D:2 * D], f2ln[:], D)
            fout = pw.tile([128, D], FP32, tag="res_t1", name="fout")
            leaky(f2ln[:], fout[:], D)
            nc.sync.dma_start(out=out_fused[t * 128:(t + 1) * 128, :], in_=fout[:])

    nc.compile()
    return nc


def make_in_maps(geo_feat, sem_feat, rsa_feat, pos, params):
    p = params
    f32 = np.float32
    def T(x):
        return np.ascontiguousarray(np.asarray(x, dtype=f32).T)

    posT = T(pos)
    brow = np.stack([np.asarray(p[k], f32) for k in
                     ("bg", "bs", "br", "bq", "bk", "bt", "bv", "bf2")])
    gb_col = np.stack([np.asarray(p[k], f32) for k in
                       ("gg", "bgn", "gs", "bsn", "gr", "brn", "gt", "btn")], axis=1)
    g1b1 = np.concatenate([np.tile(np.asarray(p[k], f32)[None, :], (128, 1))
                           for k in ("g1", "b1", "g2", "b2")], axis=1)
    gbf1 = np.concatenate([np.tile(np.asarray(p[k], f32)[None, :], (128, 1))
                           for k in ("gf1", "bf1n")], axis=1)
    gbf2 = np.concatenate([np.tile(np.asarray(p[k], f32)[None, :], (128, 1))
                           for k in ("gf2", "bf2n")], axis=1)
    bo_rep = np.tile(np.asarray(p["bo"], f32)[None, :], (128, 1))
    scal = np.tile(np.array([p["a_g"], p["be_g"], p["a_s"], p["be_s"]], f32)[None, :], (128, 1))

    shared = {
        "posT": posT,
        "Wg": np.asarray(p["Wg"], f32), "Ws": np.asarray(p["Ws"], f32), "Wr": np.asarray(p["Wr"], f32),
        "Wq": np.asarray(p["Wq"], f32), "Wk": np.asarray(p["Wk"], f32),
        "Wv": np.asarray(p["Wv"], f32), "Wo": np.asarray(p["Wo"], f32), "Wt": np.asarray(p["Wt"], f32),
        "Wf1": np.asarray(p["Wf1"], f32), "Wf2": np.asarray(p["Wf2"], f32),
        "brow": np.ascontiguousarray(brow), "bf1_row": np.asarray(p["bf1"], f32)[None, :],
        "gb_col": np.ascontiguousarray(gb_col),
        "g1b1": g1b1, "gbf1": gbf1, "gbf2": gbf2, "bo_rep": bo_rep, "scal": scal,
    }
    in_maps = []
    for c in range(N_CORES):
        r0 = c * R
        m = dict(shared)
        m["geoT"] = T(np.asarray(geo_feat)[r0:r0 + R])
        m["semT"] = T(np.asarray(sem_feat)[r0:r0 + R])
        m["rsaT"] = T(np.asarray(rsa_feat)[r0:r0 + R])
        m["posrT"] = np.ascontiguousarray(posT[:, r0:r0 + R])
        in_maps.append(m)
    return in_maps


def kernel(geo_feat, sem_feat, rsa_feat, pos, params):
    if "nc" not in _CACHE:
        _CACHE["nc"] = build_program()
    nc = _CACHE["nc"]
    in_maps = make_in_maps(geo_feat, sem_feat, rsa_feat, pos, params)

    if os.environ.get("KERNEL_SIM"):
        from concourse.bass_interp import MultiCoreSim
        sim = MultiCoreSim(nc, num_cores=N_CORES, trace=False)
        for c in range(N_CORES):
            core = sim.cores[c]
            for k, v in in_maps[c].items():
                core.tensor(k)[:] = v
        sim.simulate(check_with_hw=False)
        outs = [np.array(sim.cores[c].tensor("fused")) for c in range(N_CORES)]
    else:
        try:
            res = bass_utils.run_bass_kernel_spmd(nc, in_maps, list(range(N_CORES)))
        except Exception:
            try:
                import ctypes, jax
                jax.devices()
                ctypes.CDLL("/opt/axon/libaxon_pjrt.so").axon_reset()
                x = jax.device_put(np.ones((2, 2), np.float32), jax.devices()[0])
                np.asarray(x + 1)
            except Exception:
                pass
            res = bass_utils.run_bass_kernel_spmd(nc, in_maps, list(range(N_CORES)))
        outs = [np.asarray(res.results[c]["fused"]) for c in range(N_CORES)]
    return np.concatenate(outs, axis=0)
